# revision 31
# baseline (speedup 1.0000x reference)
"""Multi-head self-attention (shared q/k/v projection per head) + output
projection + LayerNorm, data-parallel over batch across 8 NeuronCores.

Shapes (hardcoded): B=8, S=512, E=768, H=12.
Each core handles one batch element b: full attention for all 12 heads,
the output projection, and the final LayerNorm. No collectives; the host
scatters x/mask per batch element and concatenates the 8 outputs.

Per-core dataflow (all big matmuls fp32r = full PE rate at N>=256):
  xT  = x^T                        (24 PE transposes, once)
  per head h:
    pT[e,s]    = Wh_h^T @ xT + bh  (36 MMs; bias in ACT psum->sbuf drain)
    esym[k,q]  = exp(pT^T pT/sqrt(E))  UNMASKED, symmetric: only the
                 upper-triangle k-tiles are computed (free dims
                 512/384/256/256); the 5 strictly-lower 128x128 tiles
                 are PE transposes of upper ones.  The key-pad mask is
                 applied downstream (z rows + r chain), so transposed
                 tiles need no fixup.
    rT[1,q]    = m_col^T @ esym    (4 MMs; m=1-mask); transpose+recip ->
                 recip_col[q] per-partition, ready before the y drains.
    z[s,f]     = pT^T chunks @ Wo_h (+mask scale on drain: z rows for
                 masked keys are zeroed => masked keys drop out of y)
    y[q,f]    += recip[q] * (esym[.,q]^T @ z)   (+bo on head 0)
  LayerNorm(y) * gamma + beta  -> out
"""

import math
from contextlib import ExitStack

import numpy as np

B, S, E, H = 8, 512, 768, 12
EC = E // 128  # 6 chunks of e
SC = S // 128  # 4 chunks of s
FH = 2  # f halves of 384 for z/y matmuls
FW = E // FH  # 384
EPS = 1e-5
INV_SQRT_E = 1.0 / math.sqrt(E)

# scores pass kt -> first q column computed directly (free dim >= 256
# keeps fp32r at full rate; kt=3 recomputes tile (3,2) rather than
# running a 128-free matmul at 1/4 rate)
Q0 = (0, 128, 256, 384)
# lower-triangle esym tiles produced by transposing the symmetric upper
# ones: batches emitted after the exp drain their sources depend on.
T_BATCH1 = ((1, 0), (2, 0), (3, 0))  # sources (0,1),(0,2),(0,3): pass 0
T_BATCH2 = ((2, 1), (3, 1))  # sources (1,2),(1,3): pass 1
T_BATCH3 = ((3, 2),)  # source (2,3): pass 2

_CACHE = {}


def _emit(nc, tc, tensors):
    import concourse.mybir as mybir

    F32 = mybir.dt.float32
    F32R = mybir.dt.float32r
    BF16 = mybir.dt.bfloat16
    I32 = mybir.dt.int32
    AF = mybir.ActivationFunctionType
    OP = mybir.AluOpType

    x_d, mask_d, wh_d, bh_d, wo_d, bo_d, gamma_d, beta_d, y_d = tensors

    ctx = ExitStack()
    pool = lambda name, bufs, **kw: ctx.enter_context(
        tc.tile_pool(name=name, bufs=bufs, **kw)
    )
    constp = pool("const", 1)
    xtp = pool("xt", 1)
    yp = pool("y", 1)
    # PSUM: 8 banks total. a=3 (proj/z chains), sc=3 (scores, transpose
    # scratch, rT, broadcasts), y0/y1 = 1 each.
    ps_a = pool("ps_a", 3, space="PSUM")
    ps_sc = pool("ps_sc", 3, space="PSUM")
    ps_y = pool("ps_y", 1, space="PSUM")

    whp = pool("wh", 2)
    wop = pool("wo", 2)
    wobp = pool("wob", 2)
    bhp = pool("bh", 2)
    ptp = pool("pt", 2)
    expp = pool("esym", 1)
    zp = pool("z", 1)
    smallp = pool("small", 2)
    statp = pool("stat", 16)
    lnp = pool("ln", 2)

    # ---- constants ----
    ident_d = nc.inline_tensor(np.eye(128, dtype=np.float32), name="ident128")
    ident = constp.tile([128, 128], F32R)
    nc.gpsimd.dma_start(ident[:], ident_d.ap())
    ident1 = constp.tile([1, 1], F32)
    nc.vector.memset(ident1[:], 1.0)
    # eps_t is produced via ACT Sqrt *after the last exp of head 11*
    # (emitted in head()): the Sqrt act-table set evicts/get evicted by
    # the Exp set, so the 1.3us table swap must land after the final Exp
    # but before the layernorm's first Sqrt -- otherwise it stalls the
    # LN chain, which backs up the DVE FIFO ahead of the y drains and
    # stalls the PE on PSUM bank reuse.
    eps_sq = constp.tile([128, 1], F32)
    nc.vector.memset(eps_sq[:], EPS * EPS)
    eps_t = constp.tile([128, 1], F32)

    # PE warmup: the HAM clock gate defaults to 1.2GHz and needs ~3.4us of
    # sustained matmul activity to release to 2.4GHz; the prologue is
    # DMA-bound, so without this the whole first head runs at half clock.
    # ~16 N=128 dummy matmuls on the identity keep the PE "busy" from
    # t~8us (ident is the first DMA to land) until real work streams in.
    warm = ps_y.tile([128, S], F32, tag="y0", name="warm")
    NWARM = 16
    for i in range(NWARM):
        nc.tensor.matmul(
            warm[:, :128],
            ident[:],
            ident[:],
            start=(i == 0),
            stop=(i == NWARM - 1),
        )


    m_col = constp.tile([128, SC], BF16)  # 1 - mask, per k-chunk column
    m_colf = constp.tile([128, SC], F32)  # same values, f32 for ACT/DVE scale
    ident_b = constp.tile([128, 128], BF16)  # for bf16 (esym) PE transposes
    nc.vector.tensor_copy(ident_b[:], ident[:])
    bo_row = constp.tile([1, E], F32R)
    gamma_bc = constp.tile([128, E], F32)
    beta_bc = constp.tile([128, E], F32)
    bo_bc = constp.tile([128, E], F32)
    ones_row_d = nc.inline_tensor(np.ones((1, 128), dtype=np.float32), name="ones_row")
    ones_row = constp.tile([1, 128], F32R)
    nc.gpsimd.dma_start(ones_row[:], ones_row_d.ap())

    xt = xtp.tile([128, EC * S], F32R)
    y_sb = yp.tile([128, SC * E], F32)

    def load_wh(h):
        wh = whp.tile([128, EC * E], F32R, tag="wh")
        nc.sync.dma_start(
            wh[:].rearrange("p (c e) -> p c e", c=EC),
            wh_d.ap()[h].rearrange("(c p) e -> p c e", p=128),
        )
        bh_t = bhp.tile([128, EC], F32, tag="bh")
        nc.sync.dma_start(bh_t[:], bh_d.ap()[h].rearrange("(c p) -> p c", p=128))
        return wh, bh_t

    def load_wo(h):
        wo = wop.tile([128, EC * E], F32R, tag="wo")
        nc.sync.dma_start(
            wo[:].rearrange("p (c e) -> p c e", c=EC),
            wo_d.ap()[h].rearrange("(c p) e -> p c e", p=128),
        )
        wob = wobp.tile([128, EC * E], BF16, tag="wob")
        return wo, wob

    def cast_wo(wo, wob):
        # f32->bf16 cast on ACT (z's moving operand must match bf16 pT):
        # two halves, ~1.9us each, emitted where ACT is otherwise idle
        half = EC * E // 2
        nc.scalar.copy(wob[:, :half], wo[:, :half])
        nc.scalar.copy(wob[:, half:], wo[:, half:])

    # ---- prologue: interleave head-0 Wh chunks with x slices on the DMA
    # queue; transposes + first proj matmuls start after ~650KB.
    wh0 = whp.tile([128, EC * E], F32R, tag="wh")
    bh0 = bhp.tile([128, EC], F32, tag="bh")
    xall = ptp.tile([128, SC * E], F32R, tag="pt", padded_shape=[128, SC * E])
    xv = xall[:].rearrange("p (t e) -> p t e", t=SC)
    for ic in range(EC):
        nc.sync.dma_start(
            xv[:, :, ic * 128 : (ic + 1) * 128],
            x_d.ap()
            .rearrange("(t p) e -> p t e", p=128)[:, :, ic * 128 : (ic + 1) * 128],
        )
        nc.sync.dma_start(
            wh0[:, ic * E : (ic + 1) * E],
            wh_d.ap()[0, ic * 128 : (ic + 1) * 128, :],
        )
    nc.sync.dma_start(bh0[:], bh_d.ap()[0].rearrange("(c p) -> p c", p=128))

    mask_i = statp.tile([128, SC], I32, tag="stat")
    nc.sync.dma_start(mask_i[:], mask_d.ap()[0].rearrange("(c p) -> p c", p=128))
    nc.vector.tensor_scalar(
        out=m_col[:], in0=mask_i[:], scalar1=-1.0, scalar2=1.0, op0=OP.mult, op1=OP.add
    )
    nc.vector.tensor_scalar(
        out=m_colf[:], in0=mask_i[:], scalar1=-1.0, scalar2=1.0, op0=OP.mult, op1=OP.add
    )
    nc.sync.dma_start(bo_row[:], bo_d.ap())
    gamma_row = lnp.tile([1, E], F32R, tag="lnt")
    nc.sync.dma_start(gamma_row[:], gamma_d.ap())
    beta_row = lnp.tile([1, E], F32R, tag="lnsq")
    nc.sync.dma_start(beta_row[:], beta_d.ap())

    wo0, wob0 = load_wo(0)

    # x transposes: 4 per e-chunk batched into one PSUM tile, one copy
    for ec in range(EC):
        trp = ps_sc.tile([128, S], F32R, tag="sc")
        for t in range(SC):
            nc.tensor.transpose(
                trp[:, t * 128 : (t + 1) * 128],
                xall[:, t * E + ec * 128 : t * E + ec * 128 + 128],
                ident[:],
            )
        nc.scalar.copy(xt[:, ec * S : (ec + 1) * S], trp[:])

    # broadcast gamma/beta/bo rows to all partitions via K=1 matmuls
    for row, bc in ((gamma_row, gamma_bc), (beta_row, beta_bc), (bo_row, bo_bc)):
        for f in range(FH):
            bps = ps_sc.tile([128, S], F32, tag="sc")
            nc.tensor.matmul(
                bps[:, :FW],
                ones_row[:],
                row[:, f * FW : (f + 1) * FW],
                start=True,
                stop=True,
            )
            nc.vector.tensor_copy(bc[:, f * FW : (f + 1) * FW], bps[:, :FW])

    cast_wo(wo0, wob0)

    loaded0 = (wh0, bh0, wob0)

    def proj_head0(wh, bh_t, pt):
        # ic-inner groups of 3 so the PE starts on Wh chunk 0 instead of
        # waiting for the full 2.25MB of Wh0 (prologue is DMA-bound)
        for g in range(2):
            pps = [
                ps_a.tile([128, S], F32, tag="a", name=f"pp0_{g}_{j}")
                for j in range(3)
            ]
            for ic in range(EC):
                for j in range(3):
                    et = g * 3 + j
                    nc.tensor.matmul(
                        pps[j][:],
                        wh[:, ic * E + et * 128 : ic * E + et * 128 + 128],
                        xt[:, ic * S : (ic + 1) * S],
                        start=(ic == 0),
                        stop=(ic == EC - 1),
                    )
            for j in range(3):
                et = g * 3 + j
                nc.scalar.activation(
                    pt[:, et * S : (et + 1) * S],
                    pps[j][:],
                    AF.Identity,
                    bias=bh_t[:, et : et + 1],
                    scale=1.0,
                )

    def proj(wh, bh_t, pt):
        for et in range(EC):
            pps = ps_a.tile([128, S], F32, tag="a")
            for ic in range(EC):
                nc.tensor.matmul(
                    pps[:],
                    wh[:, ic * E + et * 128 : ic * E + et * 128 + 128],
                    xt[:, ic * S : (ic + 1) * S],
                    start=(ic == 0),
                    stop=(ic == EC - 1),
                )
            nc.scalar.activation(
                pt[:, et * S : (et + 1) * S],
                pps[:],
                AF.Identity,
                bias=bh_t[:, et : et + 1],
                scale=1.0,
            )

    def transpose_batch(esym, esym_v, pairs, qlo):
        trp = ps_sc.tile([128, S], BF16, tag="sc", padded_shape=[128, 1024])
        for j, (dk, dq) in enumerate(pairs):
            nc.tensor.transpose(
                trp[:, j * 128 : (j + 1) * 128],
                esym[:, dq * S + dk * 128 : dq * S + dk * 128 + 128],
                ident_b[:],
            )
        n = len(pairs)
        k0 = pairs[0][0]
        nc.vector.tensor_copy(
            esym_v[:, k0 : k0 + n, qlo : qlo + 128],
            trp[:, : n * 128].rearrange("p (k q) -> p k q", k=n),
        )

    def ln_qt(qt, mh0, mh1):
        # layernorm of one q-tile, pipelined behind the last head's y
        # drains: mean comes free from the drains' accum_out; squares on
        # ACT, stats on DVE, the two full-width passes on the idle Pool
        # engine so nothing serializes behind the attention drains.
        ys = y_sb[:, qt * E : (qt + 1) * E]
        musum = statp.tile([128, 1], F32, tag="stat")
        nc.vector.scalar_tensor_tensor(
            out=musum[:], in0=mh0[:], scalar=0.0, in1=mh1[:], op0=OP.add, op1=OP.add
        )

        ssq = []
        for hf in range(FH):
            scr = lnp.tile([128, FW], F32, tag=("lnt", "lnsq")[hf])
            sq = statp.tile([128, 1], F32, tag="stat", name=f"ssq_{qt}_{hf}")
            nc.scalar.activation(
                scr[:],
                y_sb[:, qt * E + hf * FW : qt * E + (hf + 1) * FW],
                AF.Square,
                accum_out=sq[:],
            )
            ssq.append(sq)
        vart = statp.tile([128, 1], F32, tag="stat")
        nc.vector.tensor_scalar(
            out=vart[:],
            in0=musum[:],
            scalar1=musum[:],
            scalar2=-1.0 / (E * E),
            op0=OP.mult,
            op1=OP.mult,
        )  # vart = -mu^2
        var2a = statp.tile([128, 1], F32, tag="stat")
        nc.vector.scalar_tensor_tensor(
            out=var2a[:],
            in0=ssq[0][:],
            scalar=1.0 / E,
            in1=vart[:],
            op0=OP.mult,
            op1=OP.add,
        )
        var2 = statp.tile([128, 1], F32, tag="stat")
        nc.vector.scalar_tensor_tensor(
            out=var2[:],
            in0=ssq[1][:],
            scalar=1.0 / E,
            in1=var2a[:],
            op0=OP.mult,
            op1=OP.add,
        )  # var2 = ssq/E - mu^2
        std = statp.tile([128, 1], F32, tag="stat")
        nc.scalar.activation(std[:], var2[:], AF.Sqrt, bias=eps_t[:], scale=1.0)
        rstd = statp.tile([128, 1], F32, tag="stat")
        nc.vector.reciprocal(rstd[:], std[:])
        cc = statp.tile([128, 1], F32, tag="stat")
        nc.vector.tensor_scalar(
            out=cc[:],
            in0=musum[:],
            scalar1=rstd[:],
            scalar2=-1.0 / E,
            op0=OP.mult,
            op1=OP.mult,
        )  # cc = -mu * rstd
        # o1 = (ys - mu) * rstd on ACT (per-partition scale+bias), then
        # *gamma, +beta as plain tensor-tensor passes on the idle Pool
        # engine -- keeps the big elementwise work off DVE, which is busy
        # with the last head's y drains.
        o1 = lnp.tile([128, E], F32, tag="lnt")
        nc.scalar.activation(o1[:], ys, AF.Identity, bias=cc[:], scale=rstd[:])
        t2 = lnp.tile([128, E], F32, tag="lnsq")
        nc.vector.tensor_mul(t2[:], o1[:], gamma_bc[:])
        yout = lnp.tile([128, E], F32, tag="lnyo")
        # last qt: the +beta pass is the final critical-path op -- run it
        # on DVE (~1us) instead of the slower Pool (~1.8us)
        eng = nc.vector if qt == SC - 1 else nc.gpsimd
        eng.tensor_add(yout[:], t2[:], beta_bc[:])
        nc.sync.dma_start(y_d.ap()[qt * 128 : (qt + 1) * 128, :], yout[:])

    def head(h, loaded, nwo, nwob):
        wh, bh_t, wo = loaded

        pt = ptp.tile([128, EC * S], BF16, tag="pt")
        if h == 0:
            proj_head0(wh, bh_t, pt)
        else:
            proj(wh, bh_t, pt)

        # scores (upper triangle) + exp; lower tiles by transpose
        esym = expp.tile([128, SC * S], BF16, tag="esym")
        esym_v = esym[:].rearrange("p (k q) -> p k q", k=SC)
        for kt in range(SC):
            q0 = Q0[kt]
            fw = S - q0
            scs = ps_sc.tile([128, S], F32, tag="sc")
            for ec in range(EC):
                nc.tensor.matmul(
                    scs[:, :fw],
                    pt[:, ec * S + kt * 128 : ec * S + kt * 128 + 128],
                    pt[:, ec * S + q0 : ec * S + S],
                    start=(ec == 0),
                    stop=(ec == EC - 1),
                )
            nc.scalar.activation(
                esym[:, kt * S + q0 : (kt + 1) * S],
                scs[:, :fw],
                AF.Exp,
                scale=INV_SQRT_E,
            )
            if kt == 1:
                transpose_batch(esym, esym_v, T_BATCH1, 0)
            elif kt == 2:
                transpose_batch(esym, esym_v, T_BATCH2, 128)
            elif kt == 3:
                transpose_batch(esym, esym_v, T_BATCH3, 256)

        if h == H - 1:
            # final Exp is behind us: swap in the Sqrt act-table now (off
            # the critical path) so the layernorm never waits for it
            nc.scalar.activation(eps_t[:], eps_sq[:], AF.Sqrt)

        # rT[1,q] = m^T @ esym; transpose to per-partition recip ahead of
        # the y drains so normalization never stalls the PE
        rps = ps_sc.tile([1, S], F32, tag="sc")
        for kt in range(SC):
            nc.tensor.matmul(
                rps[:],
                m_col[:, kt : kt + 1],
                esym[:, kt * S : (kt + 1) * S],
                start=(kt == 0),
                stop=(kt == SC - 1),
            )
        r_sb = smallp.tile([1, S], F32, tag="rsb")
        nc.scalar.copy(r_sb[:], rps[:])
        rtp = ps_sc.tile([128, SC], F32, tag="sc")
        for qt in range(SC):
            nc.tensor.transpose(
                rtp[:, qt : qt + 1],
                r_sb[:, qt * 128 : (qt + 1) * 128],
                ident1[:],
            )
        rsum = smallp.tile([128, SC], F32, tag="rsum")
        nc.scalar.copy(rsum[:], rtp[:])
        recip_col = smallp.tile([128, SC], F32, tag="recip")
        nc.vector.reciprocal(recip_col[:], rsum[:])

        # z[s,f] = pT^T @ Wo (+bias-free); masked key rows zeroed on drain
        z = zp.tile([128, SC * E], BF16, tag="z")
        for st in range(SC):
            for hf in range(FH):
                zps = ps_a.tile([128, S], F32, tag="a")
                for ec in range(EC):
                    nc.tensor.matmul(
                        zps[:, :FW],
                        pt[:, ec * S + st * 128 : ec * S + st * 128 + 128],
                        wo[:, ec * E + hf * FW : ec * E + (hf + 1) * FW],
                        start=(ec == 0),
                        stop=(ec == EC - 1),
                    )
                dst = z[:, st * E + hf * FW : st * E + (hf + 1) * FW]
                if hf == 0:
                    nc.scalar.mul(dst, zps[:, :FW], m_colf[:, st : st + 1])
                else:
                    nc.vector.tensor_scalar(
                        out=dst,
                        in0=zps[:, :FW],
                        scalar1=m_colf[:, st : st + 1],
                        scalar2=None,
                        op0=OP.mult,
                    )

        # y[q,f] += recip[q] * sum_k esym[k,q] z[k,f]   (+bo on head 0)
        # on the last head the drain also emits the row-sum (accum_out)
        # for the layernorm mean, and ln_qt() is pipelined in per qt.
        if nwo is not None:
            cast_wo(nwo, nwob)

        last = h == H - 1
        mh_prev = None
        for qt in range(SC):
            rc = recip_col[:, qt : qt + 1]
            mh = []
            for hf in range(FH):
                yps = ps_y.tile([128, S], F32, tag=f"y{hf}")
                for kt in range(SC):
                    nc.tensor.matmul(
                        yps[:, :FW],
                        esym[:, kt * S + qt * 128 : kt * S + qt * 128 + 128],
                        z[:, kt * E + hf * FW : kt * E + (hf + 1) * FW],
                        start=(kt == 0),
                        stop=(kt == SC - 1),
                    )
                ysl = y_sb[:, qt * E + hf * FW : qt * E + (hf + 1) * FW]
                other = bo_bc[:, hf * FW : (hf + 1) * FW] if h == 0 else ysl
                acc = None
                if last:
                    acc = statp.tile(
                        [128, 1], F32, tag="stat", name=f"mh_{qt}_{hf}"
                    )
                    mh.append(acc)
                nc.vector.scalar_tensor_tensor(
                    out=ysl,
                    in0=yps[:, :FW],
                    scalar=rc,
                    in1=other,
                    op0=OP.mult,
                    op1=OP.add,
                    accum_out=acc,
                )
            if last:
                # lag the layernorm chain one qt behind the drains so its
                # DVE/ACT ops never sit in the engine FIFOs ahead of the
                # next qt's drains (which gate PSUM bank reuse -> PE).
                if mh_prev is not None:
                    ln_qt(qt - 1, mh_prev[0], mh_prev[1])
                mh_prev = (mh[0], mh[1])
        if last:
            ln_qt(SC - 1, mh_prev[0], mh_prev[1])

    loaded = loaded0
    for h in range(H):
        with nc.named_scope(f"head{h}"):
            if h + 1 < H:
                nwh, nbh = load_wh(h + 1)
                nwo, nwob = load_wo(h + 1)
                nxt = (nwh, nbh, nwob)
            else:
                nwo = nwob = None
                nxt = None
            head(h, loaded, nwo, nwob)
            loaded = nxt

    ctx.close()


def _build_nc():
    import concourse.bacc as bacc
    import concourse.mybir as mybir
    import concourse.tile as tile

    F32 = mybir.dt.float32
    I32 = mybir.dt.int32

    nc = bacc.Bacc("TRN2", target_bir_lowering=False, debug=False, enable_asserts=True)

    # f32r DRAM declarations: same bits as f32 (dt.np(float32r) == np.float32)
    # but lets plain HWDGE (nc.sync) DMAs feed f32r SBUF tiles without the
    # gpsimd casting path, which would serialize all weight loads on one queue.
    F32R = mybir.dt.float32r
    tensors = (
        nc.dram_tensor("x", [S, E], F32R, kind="ExternalInput"),
        nc.dram_tensor("mask", [1, S], I32, kind="ExternalInput"),
        nc.dram_tensor("wh", [H, E, E], F32R, kind="ExternalInput"),
        nc.dram_tensor("bh", [H, E], F32, kind="ExternalInput"),
        nc.dram_tensor("wo", [H, E, E], F32R, kind="ExternalInput"),
        nc.dram_tensor("bo", [1, E], F32R, kind="ExternalInput"),
        nc.dram_tensor("gamma", [1, E], F32R, kind="ExternalInput"),
        nc.dram_tensor("beta", [1, E], F32R, kind="ExternalInput"),
        nc.dram_tensor("y", [S, E], F32, kind="ExternalOutput"),
    )

    with tile.TileContext(nc) as tc:
        _emit(nc, tc, tensors)

    nc.compile()
    return nc


def get_nc():
    if "nc" not in _CACHE:
        _CACHE["nc"] = _build_nc()
    return _CACHE["nc"]


def make_in_maps(x, atten_pad_mask, Wh, bh, Wo, bo, gamma, beta):
    x = np.ascontiguousarray(np.asarray(x, dtype=np.float32))
    mask = np.ascontiguousarray(np.asarray(atten_pad_mask, dtype=np.int32))
    wh = np.ascontiguousarray(np.asarray(Wh, dtype=np.float32))
    bhv = np.ascontiguousarray(np.asarray(bh, dtype=np.float32))
    wo = np.ascontiguousarray(np.asarray(Wo, dtype=np.float32).reshape(H, E, E))
    bov = np.asarray(bo, dtype=np.float32).reshape(1, E)
    gam = np.asarray(gamma, dtype=np.float32).reshape(1, E)
    bet = np.asarray(beta, dtype=np.float32).reshape(1, E)
    return [
        {
            "x": x[b],
            "mask": mask[b],
            "wh": wh,
            "bh": bhv,
            "wo": wo,
            "bo": bov,
            "gamma": gam,
            "beta": bet,
        }
        for b in range(B)
    ]


def kernel(x, atten_pad_mask, Wh, bh, Wo, bo, gamma, beta):
    from concourse.bass_utils import run_bass_kernel_spmd

    nc = get_nc()
    in_maps = make_in_maps(x, atten_pad_mask, Wh, bh, Wo, bo, gamma, beta)
    res = run_bass_kernel_spmd(nc, in_maps, list(range(B)))
    return np.stack([res.results[b]["y"] for b in range(B)], axis=0)


# revision 32
# speedup vs baseline: 1.0032x; 1.0032x over previous
"""Multi-head self-attention (shared q/k/v projection per head) + output
projection + LayerNorm, data-parallel over batch across 8 NeuronCores.

Shapes (hardcoded): B=8, S=512, E=768, H=12.
Each core handles one batch element b: full attention for all 12 heads,
the output projection, and the final LayerNorm. No collectives; the host
scatters x/mask per batch element and concatenates the 8 outputs.

Per-core dataflow (proj matmuls fp32r; attention-side matmuls bf16 so
the stationary operand gets the compiler's fast weight load, which
floors the cadence of N<=384 matmuls at the stream rate):
  xT  = x^T                        (24 PE transposes, once)
  per head h:
    pT[e,s]    = Wh_h^T @ xT + bh  (36 MMs; bias in ACT psum->sbuf
                 drain, output cast to bf16)
    esym[k,q]  = exp(pT^T pT/sqrt(E))  UNMASKED, symmetric: only the
                 upper-triangle k-tiles are computed (free dims
                 512/384/256/128); the 6 strictly-lower 128x128 tiles
                 are PE transposes of upper ones.  The key-pad mask is
                 applied downstream (z rows + r chain), so transposed
                 tiles need no fixup.
    rT[1,q]    = m_col^T @ esym    (4 MMs; m=1-mask); transpose+recip ->
                 recip_col[q] per-partition, ready before the y drains.
    z[s,f]     = pT^T chunks @ Wo_h (+mask scale on drain: z rows for
                 masked keys are zeroed => masked keys drop out of y)
    y[q,f]    += recip[q] * (esym[.,q]^T @ z)   (+bo on head 0)
  LayerNorm(y) * gamma + beta  -> out
"""

import math
from contextlib import ExitStack

import numpy as np

B, S, E, H = 8, 512, 768, 12
EC = E // 128  # 6 chunks of e
SC = S // 128  # 4 chunks of s
FH = 2  # f halves of 384 for z/y matmuls
FW = E // FH  # 384
EPS = 1e-5
INV_SQRT_E = 1.0 / math.sqrt(E)

# scores pass kt -> first q column computed directly (free dim >= 256
# keeps fp32r at full rate; kt=3 recomputes tile (3,2) rather than
# running a 128-free matmul at 1/4 rate)
Q0 = (0, 128, 256, 384)
# lower-triangle esym tiles produced by transposing the symmetric upper
# ones: batches emitted after the exp drain their sources depend on.
T_BATCH1 = ((1, 0), (2, 0), (3, 0))  # sources (0,1),(0,2),(0,3): pass 0
T_BATCH2 = ((2, 1), (3, 1))  # sources (1,2),(1,3): pass 1
T_BATCH3 = ((3, 2),)  # source (2,3): pass 2

_CACHE = {}


def _emit(nc, tc, tensors):
    import concourse.mybir as mybir

    F32 = mybir.dt.float32
    F32R = mybir.dt.float32r
    BF16 = mybir.dt.bfloat16
    I32 = mybir.dt.int32
    AF = mybir.ActivationFunctionType
    OP = mybir.AluOpType

    x_d, mask_d, wh_d, bh_d, wo_d, bo_d, gamma_d, beta_d, y_d = tensors

    ctx = ExitStack()
    pool = lambda name, bufs, **kw: ctx.enter_context(
        tc.tile_pool(name=name, bufs=bufs, **kw)
    )
    constp = pool("const", 1)
    xtp = pool("xt", 1)
    yp = pool("y", 1)
    # PSUM: 8 banks total. a=3 (proj/z chains), sc=3 (scores, transpose
    # scratch, rT, broadcasts), y0/y1 = 1 each.
    ps_a = pool("ps_a", 3, space="PSUM")
    ps_sc = pool("ps_sc", 3, space="PSUM")
    ps_y = pool("ps_y", 1, space="PSUM")

    whp = pool("wh", 2)
    wop = pool("wo", 2)
    wobp = pool("wob", 2)
    bhp = pool("bh", 2)
    ptp = pool("pt", 2)
    expp = pool("esym", 1)
    zp = pool("z", 1)
    smallp = pool("small", 2)
    statp = pool("stat", 16)
    lnp = pool("ln", 2)

    # ---- constants ----
    ident_d = nc.inline_tensor(np.eye(128, dtype=np.float32), name="ident128")
    ident = constp.tile([128, 128], F32R)
    nc.gpsimd.dma_start(ident[:], ident_d.ap())
    ident1 = constp.tile([1, 1], F32)
    nc.vector.memset(ident1[:], 1.0)
    # eps_t is produced via ACT Sqrt *after the last exp of head 11*
    # (emitted in head()): the Sqrt act-table set evicts/get evicted by
    # the Exp set, so the 1.3us table swap must land after the final Exp
    # but before the layernorm's first Sqrt -- otherwise it stalls the
    # LN chain, which backs up the DVE FIFO ahead of the y drains and
    # stalls the PE on PSUM bank reuse.
    eps_sq = constp.tile([128, 1], F32)
    nc.vector.memset(eps_sq[:], EPS * EPS)
    eps_t = constp.tile([128, 1], F32)

    # PE warmup: the HAM clock gate defaults to 1.2GHz and needs ~3.4us of
    # sustained matmul activity to release to 2.4GHz; the prologue is
    # DMA-bound, so without this the whole first head runs at half clock.
    # ~16 N=128 dummy matmuls on the identity keep the PE "busy" from
    # t~8us (ident is the first DMA to land) until real work streams in.
    warm = ps_y.tile([128, S], F32, tag="y0", name="warm")
    NWARM = 16
    for i in range(NWARM):
        nc.tensor.matmul(
            warm[:, :128],
            ident[:],
            ident[:],
            start=(i == 0),
            stop=(i == NWARM - 1),
        )


    m_col = constp.tile([128, SC], BF16)  # 1 - mask, per k-chunk column
    m_colf = constp.tile([128, SC], F32)  # same values, f32 for ACT/DVE scale
    ident_b = constp.tile([128, 128], BF16)  # for bf16 (esym) PE transposes
    nc.vector.tensor_copy(ident_b[:], ident[:])
    bo_row = constp.tile([1, E], F32R)
    gamma_bc = constp.tile([128, E], F32)
    beta_bc = constp.tile([128, E], F32)
    bo_bc = constp.tile([128, E], F32)
    ones_row_d = nc.inline_tensor(np.ones((1, 128), dtype=np.float32), name="ones_row")
    ones_row = constp.tile([1, 128], F32R)
    nc.gpsimd.dma_start(ones_row[:], ones_row_d.ap())

    xt = xtp.tile([128, EC * S], F32R)
    y_sb = yp.tile([128, SC * E], F32)

    def load_wh(h):
        wh = whp.tile([128, EC * E], F32R, tag="wh")
        nc.sync.dma_start(
            wh[:].rearrange("p (c e) -> p c e", c=EC),
            wh_d.ap()[h].rearrange("(c p) e -> p c e", p=128),
        )
        bh_t = bhp.tile([128, EC], F32, tag="bh")
        nc.sync.dma_start(bh_t[:], bh_d.ap()[h].rearrange("(c p) -> p c", p=128))
        return wh, bh_t

    def load_wo(h):
        wo = wop.tile([128, EC * E], F32R, tag="wo")
        nc.sync.dma_start(
            wo[:].rearrange("p (c e) -> p c e", c=EC),
            wo_d.ap()[h].rearrange("(c p) e -> p c e", p=128),
        )
        wob = wobp.tile([128, EC * E], BF16, tag="wob")
        return wo, wob

    def cast_wo(wo, wob):
        # f32->bf16 cast on ACT (z's moving operand must match bf16 pT):
        # two halves, ~1.9us each, emitted where ACT is otherwise idle
        half = EC * E // 2
        nc.scalar.copy(wob[:, :half], wo[:, :half])
        nc.scalar.copy(wob[:, half:], wo[:, half:])

    # ---- prologue: interleave head-0 Wh chunks with x slices on the DMA
    # queue; transposes + first proj matmuls start after ~650KB.
    wh0 = whp.tile([128, EC * E], F32R, tag="wh")
    bh0 = bhp.tile([128, EC], F32, tag="bh")
    xall = ptp.tile([128, SC * E], F32R, tag="pt", padded_shape=[128, SC * E])
    xv = xall[:].rearrange("p (t e) -> p t e", t=SC)
    for ic in range(EC):
        nc.sync.dma_start(
            xv[:, :, ic * 128 : (ic + 1) * 128],
            x_d.ap()
            .rearrange("(t p) e -> p t e", p=128)[:, :, ic * 128 : (ic + 1) * 128],
        )
        nc.sync.dma_start(
            wh0[:, ic * E : (ic + 1) * E],
            wh_d.ap()[0, ic * 128 : (ic + 1) * 128, :],
        )
    nc.sync.dma_start(bh0[:], bh_d.ap()[0].rearrange("(c p) -> p c", p=128))

    mask_i = statp.tile([128, SC], I32, tag="stat")
    nc.sync.dma_start(mask_i[:], mask_d.ap()[0].rearrange("(c p) -> p c", p=128))
    nc.vector.tensor_scalar(
        out=m_col[:], in0=mask_i[:], scalar1=-1.0, scalar2=1.0, op0=OP.mult, op1=OP.add
    )
    nc.vector.tensor_scalar(
        out=m_colf[:], in0=mask_i[:], scalar1=-1.0, scalar2=1.0, op0=OP.mult, op1=OP.add
    )
    nc.sync.dma_start(bo_row[:], bo_d.ap())
    gamma_row = lnp.tile([1, E], F32R, tag="lnt")
    nc.sync.dma_start(gamma_row[:], gamma_d.ap())
    beta_row = lnp.tile([1, E], F32R, tag="lnsq")
    nc.sync.dma_start(beta_row[:], beta_d.ap())

    wo0, wob0 = load_wo(0)

    # x transposes: 4 per e-chunk batched into one PSUM tile, one copy
    for ec in range(EC):
        trp = ps_sc.tile([128, S], F32R, tag="sc")
        for t in range(SC):
            nc.tensor.transpose(
                trp[:, t * 128 : (t + 1) * 128],
                xall[:, t * E + ec * 128 : t * E + ec * 128 + 128],
                ident[:],
            )
        nc.scalar.copy(xt[:, ec * S : (ec + 1) * S], trp[:])

    # broadcast gamma/beta/bo rows to all partitions via K=1 matmuls
    for row, bc in ((gamma_row, gamma_bc), (beta_row, beta_bc), (bo_row, bo_bc)):
        for f in range(FH):
            bps = ps_sc.tile([128, S], F32, tag="sc")
            nc.tensor.matmul(
                bps[:, :FW],
                ones_row[:],
                row[:, f * FW : (f + 1) * FW],
                start=True,
                stop=True,
            )
            nc.vector.tensor_copy(bc[:, f * FW : (f + 1) * FW], bps[:, :FW])

    cast_wo(wo0, wob0)

    loaded0 = (wh0, bh0, wob0)

    def proj_head0(wh, bh_t, pt):
        # ic-inner groups of 3 so the PE starts on Wh chunk 0 instead of
        # waiting for the full 2.25MB of Wh0 (prologue is DMA-bound)
        for g in range(2):
            pps = [
                ps_a.tile([128, S], F32, tag="a", name=f"pp0_{g}_{j}")
                for j in range(3)
            ]
            for ic in range(EC):
                for j in range(3):
                    et = g * 3 + j
                    nc.tensor.matmul(
                        pps[j][:],
                        wh[:, ic * E + et * 128 : ic * E + et * 128 + 128],
                        xt[:, ic * S : (ic + 1) * S],
                        start=(ic == 0),
                        stop=(ic == EC - 1),
                    )
            for j in range(3):
                et = g * 3 + j
                nc.scalar.activation(
                    pt[:, et * S : (et + 1) * S],
                    pps[j][:],
                    AF.Identity,
                    bias=bh_t[:, et : et + 1],
                    scale=1.0,
                )

    def proj(wh, bh_t, pt):
        for et in range(EC):
            pps = ps_a.tile([128, S], F32, tag="a")
            for ic in range(EC):
                nc.tensor.matmul(
                    pps[:],
                    wh[:, ic * E + et * 128 : ic * E + et * 128 + 128],
                    xt[:, ic * S : (ic + 1) * S],
                    start=(ic == 0),
                    stop=(ic == EC - 1),
                )
            nc.scalar.activation(
                pt[:, et * S : (et + 1) * S],
                pps[:],
                AF.Identity,
                bias=bh_t[:, et : et + 1],
                scale=1.0,
            )

    def transpose_batch(esym, esym_v, pairs, qlo):
        trp = ps_sc.tile([128, S], BF16, tag="sc", padded_shape=[128, 1024])
        for j, (dk, dq) in enumerate(pairs):
            nc.tensor.transpose(
                trp[:, j * 128 : (j + 1) * 128],
                esym[:, dq * S + dk * 128 : dq * S + dk * 128 + 128],
                ident_b[:],
            )
        n = len(pairs)
        k0 = pairs[0][0]
        nc.vector.tensor_copy(
            esym_v[:, k0 : k0 + n, qlo : qlo + 128],
            trp[:, : n * 128].rearrange("p (k q) -> p k q", k=n),
        )

    def ln_qt(qt, mh0, mh1):
        # layernorm of one q-tile, pipelined behind the last head's y
        # drains: mean comes free from the drains' accum_out; squares on
        # ACT, stats on DVE, the two full-width passes on the idle Pool
        # engine so nothing serializes behind the attention drains.
        ys = y_sb[:, qt * E : (qt + 1) * E]
        musum = statp.tile([128, 1], F32, tag="stat")
        nc.vector.scalar_tensor_tensor(
            out=musum[:], in0=mh0[:], scalar=0.0, in1=mh1[:], op0=OP.add, op1=OP.add
        )

        ssq = []
        for hf in range(FH):
            scr = lnp.tile([128, FW], F32, tag=("lnt", "lnsq")[hf])
            sq = statp.tile([128, 1], F32, tag="stat", name=f"ssq_{qt}_{hf}")
            nc.scalar.activation(
                scr[:],
                y_sb[:, qt * E + hf * FW : qt * E + (hf + 1) * FW],
                AF.Square,
                accum_out=sq[:],
            )
            ssq.append(sq)
        vart = statp.tile([128, 1], F32, tag="stat")
        nc.vector.tensor_scalar(
            out=vart[:],
            in0=musum[:],
            scalar1=musum[:],
            scalar2=-1.0 / (E * E),
            op0=OP.mult,
            op1=OP.mult,
        )  # vart = -mu^2
        var2a = statp.tile([128, 1], F32, tag="stat")
        nc.vector.scalar_tensor_tensor(
            out=var2a[:],
            in0=ssq[0][:],
            scalar=1.0 / E,
            in1=vart[:],
            op0=OP.mult,
            op1=OP.add,
        )
        var2 = statp.tile([128, 1], F32, tag="stat")
        nc.vector.scalar_tensor_tensor(
            out=var2[:],
            in0=ssq[1][:],
            scalar=1.0 / E,
            in1=var2a[:],
            op0=OP.mult,
            op1=OP.add,
        )  # var2 = ssq/E - mu^2
        std = statp.tile([128, 1], F32, tag="stat")
        nc.scalar.activation(std[:], var2[:], AF.Sqrt, bias=eps_t[:], scale=1.0)
        rstd = statp.tile([128, 1], F32, tag="stat")
        nc.vector.reciprocal(rstd[:], std[:])
        cc = statp.tile([128, 1], F32, tag="stat")
        nc.vector.tensor_scalar(
            out=cc[:],
            in0=musum[:],
            scalar1=rstd[:],
            scalar2=-1.0 / E,
            op0=OP.mult,
            op1=OP.mult,
        )  # cc = -mu * rstd
        # o1 = (ys - mu) * rstd on ACT (per-partition scale+bias), then
        # *gamma, +beta as plain tensor-tensor passes on the idle Pool
        # engine -- keeps the big elementwise work off DVE, which is busy
        # with the last head's y drains.
        o1 = lnp.tile([128, E], F32, tag="lnt")
        nc.scalar.activation(o1[:], ys, AF.Identity, bias=cc[:], scale=rstd[:])
        t2 = lnp.tile([128, E], F32, tag="lnsq")
        nc.vector.tensor_mul(t2[:], o1[:], gamma_bc[:])
        yout = lnp.tile([128, E], F32, tag="lnyo")
        # last qt: the +beta pass is the final critical-path op -- run it
        # on DVE (~1us) instead of the slower Pool (~1.8us)
        eng = nc.vector if qt == SC - 1 else nc.gpsimd
        eng.tensor_add(yout[:], t2[:], beta_bc[:])
        nc.sync.dma_start(y_d.ap()[qt * 128 : (qt + 1) * 128, :], yout[:])

    def head(h, loaded, nwo, nwob):
        wh, bh_t, wo = loaded

        pt = ptp.tile([128, EC * S], BF16, tag="pt")
        if h == 0:
            proj_head0(wh, bh_t, pt)
        else:
            proj(wh, bh_t, pt)

        # scores (upper triangle) + exp; lower tiles by transpose
        esym = expp.tile([128, SC * S], BF16, tag="esym")
        esym_v = esym[:].rearrange("p (k q) -> p k q", k=SC)
        for kt in range(SC):
            q0 = Q0[kt]
            fw = S - q0
            scs = ps_sc.tile([128, S], F32, tag="sc")
            for ec in range(EC):
                nc.tensor.matmul(
                    scs[:, :fw],
                    pt[:, ec * S + kt * 128 : ec * S + kt * 128 + 128],
                    pt[:, ec * S + q0 : ec * S + S],
                    start=(ec == 0),
                    stop=(ec == EC - 1),
                )
            nc.scalar.activation(
                esym[:, kt * S + q0 : (kt + 1) * S],
                scs[:, :fw],
                AF.Exp,
                scale=INV_SQRT_E,
            )
            if kt == 1:
                transpose_batch(esym, esym_v, T_BATCH1, 0)
            elif kt == 2:
                transpose_batch(esym, esym_v, T_BATCH2, 128)
            elif kt == 3:
                transpose_batch(esym, esym_v, T_BATCH3, 256)

        if h == H - 1:
            # final Exp is behind us: swap in the Sqrt act-table now (off
            # the critical path) so the layernorm never waits for it
            nc.scalar.activation(eps_t[:], eps_sq[:], AF.Sqrt)

        # rT[1,q] = m^T @ esym; transpose to per-partition recip ahead of
        # the y drains so normalization never stalls the PE
        rps = ps_sc.tile([1, S], F32, tag="sc")
        for kt in range(SC):
            nc.tensor.matmul(
                rps[:],
                m_col[:, kt : kt + 1],
                esym[:, kt * S : (kt + 1) * S],
                start=(kt == 0),
                stop=(kt == SC - 1),
            )
        r_sb = smallp.tile([1, S], F32, tag="rsb")
        nc.scalar.copy(r_sb[:], rps[:])
        rtp = ps_sc.tile([128, SC], F32, tag="sc")
        for qt in range(SC):
            nc.tensor.transpose(
                rtp[:, qt : qt + 1],
                r_sb[:, qt * 128 : (qt + 1) * 128],
                ident1[:],
            )
        rsum = smallp.tile([128, SC], F32, tag="rsum")
        nc.scalar.copy(rsum[:], rtp[:])
        recip_col = smallp.tile([128, SC], F32, tag="recip")
        nc.vector.reciprocal(recip_col[:], rsum[:])

        # z[s,f] = pT^T @ Wo (+bias-free); masked key rows zeroed on drain
        z = zp.tile([128, SC * E], BF16, tag="z")
        for st in range(SC):
            for hf in range(FH):
                zps = ps_a.tile([128, S], F32, tag="a")
                for ec in range(EC):
                    nc.tensor.matmul(
                        zps[:, :FW],
                        pt[:, ec * S + st * 128 : ec * S + st * 128 + 128],
                        wo[:, ec * E + hf * FW : ec * E + (hf + 1) * FW],
                        start=(ec == 0),
                        stop=(ec == EC - 1),
                    )
                dst = z[:, st * E + hf * FW : st * E + (hf + 1) * FW]
                if hf == 0:
                    nc.scalar.mul(dst, zps[:, :FW], m_colf[:, st : st + 1])
                else:
                    nc.vector.tensor_scalar(
                        out=dst,
                        in0=zps[:, :FW],
                        scalar1=m_colf[:, st : st + 1],
                        scalar2=None,
                        op0=OP.mult,
                    )

        # y[q,f] += recip[q] * sum_k esym[k,q] z[k,f]   (+bo on head 0)
        # on the last head the drain also emits the row-sum (accum_out)
        # for the layernorm mean, and ln_qt() is pipelined in per qt.
        if nwo is not None:
            cast_wo(nwo, nwob)

        last = h == H - 1
        mh_prev = None
        for qt in range(SC):
            rc = recip_col[:, qt : qt + 1]
            mh = []
            for hf in range(FH):
                yps = ps_y.tile([128, S], F32, tag=f"y{hf}")
                for kt in range(SC):
                    nc.tensor.matmul(
                        yps[:, :FW],
                        esym[:, kt * S + qt * 128 : kt * S + qt * 128 + 128],
                        z[:, kt * E + hf * FW : kt * E + (hf + 1) * FW],
                        start=(kt == 0),
                        stop=(kt == SC - 1),
                    )
                ysl = y_sb[:, qt * E + hf * FW : qt * E + (hf + 1) * FW]
                other = bo_bc[:, hf * FW : (hf + 1) * FW] if h == 0 else ysl
                acc = None
                if last:
                    acc = statp.tile(
                        [128, 1], F32, tag="stat", name=f"mh_{qt}_{hf}"
                    )
                    mh.append(acc)
                nc.vector.scalar_tensor_tensor(
                    out=ysl,
                    in0=yps[:, :FW],
                    scalar=rc,
                    in1=other,
                    op0=OP.mult,
                    op1=OP.add,
                    accum_out=acc,
                )
            if last:
                # lag the layernorm chain one qt behind the drains so its
                # DVE/ACT ops never sit in the engine FIFOs ahead of the
                # next qt's drains (which gate PSUM bank reuse -> PE).
                if mh_prev is not None:
                    ln_qt(qt - 1, mh_prev[0], mh_prev[1])
                mh_prev = (mh[0], mh[1])
        if last:
            ln_qt(SC - 1, mh_prev[0], mh_prev[1])

    loaded = loaded0
    for h in range(H):
        with nc.named_scope(f"head{h}"):
            if h + 1 < H:
                nwh, nbh = load_wh(h + 1)
                nwo, nwob = load_wo(h + 1)
                nxt = (nwh, nbh, nwob)
            else:
                nwo = nwob = None
                nxt = None
            head(h, loaded, nwo, nwob)
            loaded = nxt

    ctx.close()


def _build_nc():
    import concourse.bacc as bacc
    import concourse.mybir as mybir
    import concourse.tile as tile

    F32 = mybir.dt.float32
    I32 = mybir.dt.int32

    nc = bacc.Bacc("TRN2", target_bir_lowering=False, debug=False, enable_asserts=True)

    # f32r DRAM declarations: same bits as f32 (dt.np(float32r) == np.float32)
    # but lets plain HWDGE (nc.sync) DMAs feed f32r SBUF tiles without the
    # gpsimd casting path, which would serialize all weight loads on one queue.
    F32R = mybir.dt.float32r
    tensors = (
        nc.dram_tensor("x", [S, E], F32R, kind="ExternalInput"),
        nc.dram_tensor("mask", [1, S], I32, kind="ExternalInput"),
        nc.dram_tensor("wh", [H, E, E], F32R, kind="ExternalInput"),
        nc.dram_tensor("bh", [H, E], F32, kind="ExternalInput"),
        nc.dram_tensor("wo", [H, E, E], F32R, kind="ExternalInput"),
        nc.dram_tensor("bo", [1, E], F32R, kind="ExternalInput"),
        nc.dram_tensor("gamma", [1, E], F32R, kind="ExternalInput"),
        nc.dram_tensor("beta", [1, E], F32R, kind="ExternalInput"),
        nc.dram_tensor("y", [S, E], F32, kind="ExternalOutput"),
    )

    with tile.TileContext(nc) as tc:
        _emit(nc, tc, tensors)

    nc.compile()
    return nc


def get_nc():
    if "nc" not in _CACHE:
        _CACHE["nc"] = _build_nc()
    return _CACHE["nc"]


def make_in_maps(x, atten_pad_mask, Wh, bh, Wo, bo, gamma, beta):
    x = np.ascontiguousarray(np.asarray(x, dtype=np.float32))
    mask = np.ascontiguousarray(np.asarray(atten_pad_mask, dtype=np.int32))
    wh = np.ascontiguousarray(np.asarray(Wh, dtype=np.float32))
    bhv = np.ascontiguousarray(np.asarray(bh, dtype=np.float32))
    wo = np.ascontiguousarray(np.asarray(Wo, dtype=np.float32).reshape(H, E, E))
    bov = np.asarray(bo, dtype=np.float32).reshape(1, E)
    gam = np.asarray(gamma, dtype=np.float32).reshape(1, E)
    bet = np.asarray(beta, dtype=np.float32).reshape(1, E)
    return [
        {
            "x": x[b],
            "mask": mask[b],
            "wh": wh,
            "bh": bhv,
            "wo": wo,
            "bo": bov,
            "gamma": gam,
            "beta": bet,
        }
        for b in range(B)
    ]


def kernel(x, atten_pad_mask, Wh, bh, Wo, bo, gamma, beta):
    from concourse.bass_utils import run_bass_kernel_spmd

    nc = get_nc()
    in_maps = make_in_maps(x, atten_pad_mask, Wh, bh, Wo, bo, gamma, beta)
    res = run_bass_kernel_spmd(nc, in_maps, list(range(B)))
    return np.stack([res.results[b]["y"] for b in range(B)], axis=0)


# revision 38
# speedup vs baseline: 1.0519x; 1.0486x over previous
"""Multi-head self-attention (shared q/k/v projection per head) + output
projection + LayerNorm, data-parallel over batch across 8 NeuronCores.

Shapes (hardcoded): B=8, S=512, E=768, H=12.
Each core handles one batch element b: full attention for all 12 heads,
the output projection, and the final LayerNorm. No collectives; the host
scatters x/mask per batch element and concatenates the 8 outputs.

Per-core dataflow (proj matmuls fp32r; attention-side matmuls bf16 so
the stationary operand gets the compiler's fast weight load, which
floors the cadence of N<=384 matmuls at the stream rate):
  xT  = x^T                        (24 PE transposes, once)
  per head h:
    pT[e,s]    = Wh_h^T @ xT + bh  (36 MMs; bias in ACT psum->sbuf
                 drain, output cast to bf16)
    esym[k,q]  = exp(pT^T pT/sqrt(E))  UNMASKED, symmetric: only the
                 upper-triangle k-tiles are computed (free dims
                 512/384/256/128); the 6 strictly-lower 128x128 tiles
                 are PE transposes of upper ones.  The key-pad mask is
                 applied downstream (z rows + r chain), so transposed
                 tiles need no fixup.
    rT[1,q]    = m_col^T @ esym    (4 MMs; m=1-mask); transpose+recip ->
                 recip_col[q] per-partition, ready before the y drains.
    z[s,f]     = pT^T chunks @ Wo_h (+mask scale on drain: z rows for
                 masked keys are zeroed => masked keys drop out of y)
    y[q,f]    += recip[q] * (esym[.,q]^T @ z)   (+bo on head 0)
  LayerNorm(y) * gamma + beta  -> out
"""

import math
from contextlib import ExitStack

import numpy as np

B, S, E, H = 8, 512, 768, 12
EC = E // 128  # 6 chunks of e
SC = S // 128  # 4 chunks of s
FH = 2  # f halves of 384 for z/y matmuls
FW = E // FH  # 384
EPS = 1e-5
INV_SQRT_E = 1.0 / math.sqrt(E)

# scores pass kt -> first q column computed directly (free dim >= 256
# keeps fp32r at full rate; kt=3 recomputes tile (3,2) rather than
# running a 128-free matmul at 1/4 rate)
Q0 = (0, 128, 256, 384)
# lower-triangle esym tiles produced by transposing the symmetric upper
# ones: batches emitted after the exp drain their sources depend on.
T_BATCH1 = ((1, 0), (2, 0), (3, 0))  # sources (0,1),(0,2),(0,3): pass 0
T_BATCH2 = ((2, 1), (3, 1))  # sources (1,2),(1,3): pass 1
T_BATCH3 = ((3, 2),)  # source (2,3): pass 2

_CACHE = {}


def _emit(nc, tc, tensors, trivial_gb):
    import concourse.mybir as mybir

    F32 = mybir.dt.float32
    F32R = mybir.dt.float32r
    BF16 = mybir.dt.bfloat16
    I32 = mybir.dt.int32
    AF = mybir.ActivationFunctionType
    OP = mybir.AluOpType

    x_d, mask_d, wh_d, bh_d, wo_d, bo_d, gamma_d, beta_d, y_d = tensors

    ctx = ExitStack()
    pool = lambda name, bufs, **kw: ctx.enter_context(
        tc.tile_pool(name=name, bufs=bufs, **kw)
    )
    constp = pool("const", 1)
    xtp = pool("xt", 1)
    yp = pool("y", 1)
    # PSUM: 8 banks total. a=3 (proj/z chains), sc=3 (scores, transpose
    # scratch, rT, broadcasts), y0/y1 = 1 each.
    ps_a = pool("ps_a", 3, space="PSUM")
    ps_sc = pool("ps_sc", 3, space="PSUM")
    ps_y = pool("ps_y", 1, space="PSUM")

    whp = pool("wh", 2)
    wop = pool("wo", 2)
    wobp = pool("wob", 2)
    bhp = pool("bh", 2)
    ptp = pool("pt", 2)
    expp = pool("esym", 1)
    zp = pool("z", 1)
    smallp = pool("small", 2)
    statp = pool("stat", 16)
    lnp = pool("ln", 2)

    # ---- constants ----
    ident_d = nc.inline_tensor(np.eye(128, dtype=np.float32), name="ident128")
    ident = constp.tile([128, 128], F32R)
    nc.gpsimd.dma_start(ident[:], ident_d.ap())
    ident1 = constp.tile([1, 1], F32)
    nc.vector.memset(ident1[:], 1.0)
    # eps_t is produced via ACT Sqrt *after the last exp of head 11*
    # (emitted in head()): the Sqrt act-table set evicts/get evicted by
    # the Exp set, so the 1.3us table swap must land after the final Exp
    # but before the layernorm's first Sqrt -- otherwise it stalls the
    # LN chain, which backs up the DVE FIFO ahead of the y drains and
    # stalls the PE on PSUM bank reuse.
    eps_sq = constp.tile([128, 1], F32)
    nc.vector.memset(eps_sq[:], EPS * EPS)
    eps_t = constp.tile([128, 1], F32)

    # PE warmup: the HAM clock gate defaults to 1.2GHz and needs ~3.4us of
    # sustained matmul activity to release to 2.4GHz; the prologue is
    # DMA-bound, so without this the whole first head runs at half clock.
    # ~16 N=128 dummy matmuls on the identity keep the PE "busy" from
    # t~8us (ident is the first DMA to land) until real work streams in.
    warm_src = constp.tile([128, 128], F32)
    nc.vector.memset(warm_src[:], 1.0)
    warm = ps_y.tile([128, S], F32, tag="y0", name="warm")
    NWARM = 16
    for i in range(NWARM):
        nc.tensor.matmul(
            warm[:, :128],
            warm_src[:],
            warm_src[:],
            start=(i == 0),
            stop=(i == NWARM - 1),
        )


    m_col = constp.tile([128, SC], BF16)  # 1 - mask, per k-chunk column
    m_colf = constp.tile([128, SC], F32)  # same values, f32 for ACT/DVE scale
    ident_b = constp.tile([128, 128], BF16)  # for bf16 (esym) PE transposes
    nc.vector.tensor_copy(ident_b[:], ident[:])
    bo_row = constp.tile([1, E], F32R)
    gamma_bc = constp.tile([128, E], F32)
    beta_bc = constp.tile([128, E], F32)
    bo_bc = constp.tile([128, E], F32)
    ones_row_d = nc.inline_tensor(np.ones((1, 128), dtype=np.float32), name="ones_row")
    ones_row = constp.tile([1, 128], F32R)
    nc.gpsimd.dma_start(ones_row[:], ones_row_d.ap())

    xt = xtp.tile([128, EC * S], F32R)
    y_sb = yp.tile([128, SC * E], F32)

    def load_wh(h):
        wh = whp.tile([128, EC * E], F32R, tag="wh")
        nc.sync.dma_start(
            wh[:].rearrange("p (c e) -> p c e", c=EC),
            wh_d.ap()[h].rearrange("(c p) e -> p c e", p=128),
        )
        bh_t = bhp.tile([128, EC], F32, tag="bh")
        nc.sync.dma_start(bh_t[:], bh_d.ap()[h].rearrange("(c p) -> p c", p=128))
        return wh, bh_t

    def load_wo(h):
        wo = wop.tile([128, EC * E], F32R, tag="wo")
        nc.sync.dma_start(
            wo[:].rearrange("p (c e) -> p c e", c=EC),
            wo_d.ap()[h].rearrange("(c p) e -> p c e", p=128),
        )
        wob = wobp.tile([128, EC * E], BF16, tag="wob")
        return wo, wob

    def cast_wo(wo, wob):
        # f32->bf16 cast on ACT (z's moving operand must match bf16 pT):
        # two halves, ~1.9us each, emitted where ACT is otherwise idle
        half = EC * E // 2
        nc.scalar.copy(wob[:, :half], wo[:, :half])
        nc.scalar.copy(wob[:, half:], wo[:, half:])

    # ---- prologue: interleave head-0 Wh chunks with x slices on the DMA
    # queue; transposes + first proj matmuls start after ~650KB.
    wh0 = whp.tile([128, EC * E], F32R, tag="wh")
    bh0 = bhp.tile([128, EC], F32, tag="bh")
    xall = ptp.tile([128, SC * E], F32R, tag="pt", padded_shape=[128, SC * E])
    xv = xall[:].rearrange("p (t e) -> p t e", t=SC)
    for ic in range(EC):
        nc.sync.dma_start(
            xv[:, :, ic * 128 : (ic + 1) * 128],
            x_d.ap()
            .rearrange("(t p) e -> p t e", p=128)[:, :, ic * 128 : (ic + 1) * 128],
        )
        nc.sync.dma_start(
            wh0[:, ic * E : (ic + 1) * E],
            wh_d.ap()[0, ic * 128 : (ic + 1) * 128, :],
        )
    nc.sync.dma_start(bh0[:], bh_d.ap()[0].rearrange("(c p) -> p c", p=128))

    mask_i = statp.tile([128, SC], I32, tag="stat")
    nc.sync.dma_start(mask_i[:], mask_d.ap()[0].rearrange("(c p) -> p c", p=128))
    nc.vector.tensor_scalar(
        out=m_col[:], in0=mask_i[:], scalar1=-1.0, scalar2=1.0, op0=OP.mult, op1=OP.add
    )
    nc.vector.tensor_scalar(
        out=m_colf[:], in0=mask_i[:], scalar1=-1.0, scalar2=1.0, op0=OP.mult, op1=OP.add
    )
    nc.sync.dma_start(bo_row[:], bo_d.ap())
    gamma_row = lnp.tile([1, E], F32R, tag="lnt")
    nc.sync.dma_start(gamma_row[:], gamma_d.ap())
    beta_row = lnp.tile([1, E], F32R, tag="lnsq")
    nc.sync.dma_start(beta_row[:], beta_d.ap())

    wo0 = wop.tile([128, EC * E], F32R, tag="wo")
    wob0 = wobp.tile([128, EC * E], BF16, tag="wob")
    wo0v = wo0[:].rearrange("p (c e) -> p c e", c=EC)
    wo0s = wo_d.ap()[0].rearrange("(c p) e -> p c e", p=128)
    for hhalf in range(2):
        c0, c1 = hhalf * 3, (hhalf + 1) * 3
        nc.sync.dma_start(wo0v[:, c0:c1, :], wo0s[:, c0:c1, :])
        nc.scalar.copy(wob0[:, c0 * E : c1 * E], wo0[:, c0 * E : c1 * E])

    # x transposes: 4 per e-chunk batched into one PSUM tile, one copy
    # (scratch from ps_y, leaving all 3 ps_sc banks for head-0's proj)
    for ec in range(EC):
        trp = ps_y.tile([128, S], F32R, tag="y1", name=f"xtr{ec}")
        for t in range(SC):
            nc.tensor.transpose(
                trp[:, t * 128 : (t + 1) * 128],
                xall[:, t * E + ec * 128 : t * E + ec * 128 + 128],
                ident[:],
            )
        nc.scalar.copy(xt[:, ec * S : (ec + 1) * S], trp[:])

    # broadcast rows to all partitions via K=1 matmuls (gamma/beta only
    # on the general path; the trivial-gb variant never reads them)
    bcast = [(bo_row, bo_bc)]
    if not trivial_gb:
        bcast += [(gamma_row, gamma_bc), (beta_row, beta_bc)]
    for row, bc in bcast:
        for f in range(FH):
            bps = ps_y.tile([128, S], F32, tag="y1", name=f"bps_{f}")
            nc.tensor.matmul(
                bps[:, :FW],
                ones_row[:],
                row[:, f * FW : (f + 1) * FW],
                start=True,
                stop=True,
            )
            nc.vector.tensor_copy(bc[:, f * FW : (f + 1) * FW], bps[:, :FW])

    loaded0 = (wh0, bh0, wob0)

    def proj_head0(wh, bh_t, pt):
        # all 6 et chains ic-inner (3 PSUM banks from ps_a + 3 borrowed
        # from ps_sc): every Wh0 chunk arrival unlocks 6 matmuls, so the
        # PE tracks the DMA stream instead of idling 35% per window
        pps = [
            ps_a.tile([128, S], F32, tag="a", name=f"pp0a_{j}") for j in range(3)
        ] + [
            ps_sc.tile([128, S], F32, tag="sc", name=f"pp0b_{j}") for j in range(3)
        ]
        for ic in range(EC):
            for et in range(EC):
                nc.tensor.matmul(
                    pps[et][:],
                    wh[:, ic * E + et * 128 : ic * E + et * 128 + 128],
                    xt[:, ic * S : (ic + 1) * S],
                    start=(ic == 0),
                    stop=(ic == EC - 1),
                )
        for et in range(EC):
            nc.scalar.activation(
                pt[:, et * S : (et + 1) * S],
                pps[et][:],
                AF.Identity,
                bias=bh_t[:, et : et + 1],
                scale=1.0,
            )

    def proj(wh, bh_t, pt):
        for et in range(EC):
            pps = ps_a.tile([128, S], F32, tag="a")
            for ic in range(EC):
                nc.tensor.matmul(
                    pps[:],
                    wh[:, ic * E + et * 128 : ic * E + et * 128 + 128],
                    xt[:, ic * S : (ic + 1) * S],
                    start=(ic == 0),
                    stop=(ic == EC - 1),
                )
            nc.scalar.activation(
                pt[:, et * S : (et + 1) * S],
                pps[:],
                AF.Identity,
                bias=bh_t[:, et : et + 1],
                scale=1.0,
            )

    def transpose_batch(esym, esym_v, pairs, qlo):
        trp = ps_sc.tile([128, S], BF16, tag="sc", padded_shape=[128, 1024])
        for j, (dk, dq) in enumerate(pairs):
            nc.tensor.transpose(
                trp[:, j * 128 : (j + 1) * 128],
                esym[:, dq * S + dk * 128 : dq * S + dk * 128 + 128],
                ident_b[:],
            )
        n = len(pairs)
        k0 = pairs[0][0]
        nc.vector.tensor_copy(
            esym_v[:, k0 : k0 + n, qlo : qlo + 128],
            trp[:, : n * 128].rearrange("p (k q) -> p k q", k=n),
        )

    def ln_qt(qt, mh0, mh1):
        # layernorm of one q-tile, pipelined behind the last head's y
        # drains: mean comes free from the drains' accum_out; squares on
        # ACT, stats on DVE, the two full-width passes on the idle Pool
        # engine so nothing serializes behind the attention drains.
        ys = y_sb[:, qt * E : (qt + 1) * E]
        musum = statp.tile([128, 1], F32, tag="stat")
        nc.vector.scalar_tensor_tensor(
            out=musum[:], in0=mh0[:], scalar=0.0, in1=mh1[:], op0=OP.add, op1=OP.add
        )

        ssq = []
        for hf in range(FH):
            scr = lnp.tile([128, FW], F32, tag=("lnt", "lnsq")[hf])
            sq = statp.tile([128, 1], F32, tag="stat", name=f"ssq_{qt}_{hf}")
            nc.scalar.activation(
                scr[:],
                y_sb[:, qt * E + hf * FW : qt * E + (hf + 1) * FW],
                AF.Square,
                accum_out=sq[:],
            )
            ssq.append(sq)
        vart = statp.tile([128, 1], F32, tag="stat")
        # float immediate + two tensor reads: the TensorScalarPtr form
        # (scalar1=musum AP) costs 1.5us on DVE (per-partition pointer
        # gather on the same tile); this stt form costs ~150ns.
        nc.vector.scalar_tensor_tensor(
            out=vart[:],
            in0=musum[:],
            scalar=-1.0 / (E * E),
            in1=musum[:],
            op0=OP.mult,
            op1=OP.mult,
        )  # vart = -mu^2
        var2a = statp.tile([128, 1], F32, tag="stat")
        nc.vector.scalar_tensor_tensor(
            out=var2a[:],
            in0=ssq[0][:],
            scalar=1.0 / E,
            in1=vart[:],
            op0=OP.mult,
            op1=OP.add,
        )
        var2 = statp.tile([128, 1], F32, tag="stat")
        nc.vector.scalar_tensor_tensor(
            out=var2[:],
            in0=ssq[1][:],
            scalar=1.0 / E,
            in1=var2a[:],
            op0=OP.mult,
            op1=OP.add,
        )  # var2 = ssq/E - mu^2
        std = statp.tile([128, 1], F32, tag="stat")
        nc.scalar.activation(std[:], var2[:], AF.Sqrt, bias=eps_t[:], scale=1.0)
        rstd = statp.tile([128, 1], F32, tag="stat")
        nc.vector.reciprocal(rstd[:], std[:])
        cc = statp.tile([128, 1], F32, tag="stat")
        nc.vector.tensor_scalar(
            out=cc[:],
            in0=musum[:],
            scalar1=rstd[:],
            scalar2=-1.0 / E,
            op0=OP.mult,
            op1=OP.mult,
        )  # cc = -mu * rstd
        # o1 = (ys - mu) * rstd on ACT (per-partition scale+bias), then
        # *gamma, +beta as plain tensor-tensor passes on the idle Pool
        # engine -- keeps the big elementwise work off DVE, which is busy
        # with the last head's y drains.
        o1 = lnp.tile([128, E], F32, tag="lnt")
        nc.scalar.activation(o1[:], ys, AF.Identity, bias=cc[:], scale=rstd[:])
        if trivial_gb:
            # gamma == 1 and beta == 0 (host-checked): o1 is the output
            nc.sync.dma_start(y_d.ap()[qt * 128 : (qt + 1) * 128, :], o1[:])
            return
        t2 = lnp.tile([128, E], F32, tag="lnsq")
        nc.vector.tensor_mul(t2[:], o1[:], gamma_bc[:])
        yout = lnp.tile([128, E], F32, tag="lnyo")
        # last qt: the +beta pass is the final critical-path op -- run it
        # on DVE (~1us) instead of the slower Pool (~1.8us)
        eng = nc.vector if qt == SC - 1 else nc.gpsimd
        eng.tensor_add(yout[:], t2[:], beta_bc[:])
        nc.sync.dma_start(y_d.ap()[qt * 128 : (qt + 1) * 128, :], yout[:])

    def head(h, loaded, nwo, nwob):
        wh, bh_t, wo = loaded

        pt = ptp.tile([128, EC * S], BF16, tag="pt")
        if h == 0:
            proj_head0(wh, bh_t, pt)
        else:
            proj(wh, bh_t, pt)

        # scores (upper triangle) + exp; lower tiles by transpose
        esym = expp.tile([128, SC * S], BF16, tag="esym")
        esym_v = esym[:].rearrange("p (k q) -> p k q", k=SC)
        for kt in range(SC):
            q0 = Q0[kt]
            fw = S - q0
            scs = ps_sc.tile([128, S], F32, tag="sc")
            for ec in range(EC):
                nc.tensor.matmul(
                    scs[:, :fw],
                    pt[:, ec * S + kt * 128 : ec * S + kt * 128 + 128],
                    pt[:, ec * S + q0 : ec * S + S],
                    start=(ec == 0),
                    stop=(ec == EC - 1),
                )
            nc.scalar.activation(
                esym[:, kt * S + q0 : (kt + 1) * S],
                scs[:, :fw],
                AF.Exp,
                scale=INV_SQRT_E,
            )
            if kt == 1:
                transpose_batch(esym, esym_v, T_BATCH1, 0)
            elif kt == 2:
                transpose_batch(esym, esym_v, T_BATCH2, 128)
            elif kt == 3:
                transpose_batch(esym, esym_v, T_BATCH3, 256)

        if h == H - 1:
            # final Exp is behind us: swap in the Sqrt act-table now (off
            # the critical path) so the layernorm never waits for it
            nc.scalar.activation(eps_t[:], eps_sq[:], AF.Sqrt)

        # rT[1,q] = m^T @ esym; transpose to per-partition recip ahead of
        # the y drains so normalization never stalls the PE
        rps = ps_sc.tile([1, S], F32, tag="sc")
        for kt in range(SC):
            nc.tensor.matmul(
                rps[:],
                m_col[:, kt : kt + 1],
                esym[:, kt * S : (kt + 1) * S],
                start=(kt == 0),
                stop=(kt == SC - 1),
            )
        r_sb = smallp.tile([1, S], F32, tag="rsb")
        nc.scalar.copy(r_sb[:], rps[:])
        rtp = ps_sc.tile([128, SC], F32, tag="sc")
        for qt in range(SC):
            nc.tensor.transpose(
                rtp[:, qt : qt + 1],
                r_sb[:, qt * 128 : (qt + 1) * 128],
                ident1[:],
            )
        rsum = smallp.tile([128, SC], F32, tag="rsum")
        nc.scalar.copy(rsum[:], rtp[:])
        recip_col = smallp.tile([128, SC], F32, tag="recip")
        nc.vector.reciprocal(recip_col[:], rsum[:])

        # z[s,f] = pT^T @ Wo (+bias-free); masked key rows zeroed on drain
        z = zp.tile([128, SC * E], BF16, tag="z")
        for st in range(SC):
            for hf in range(FH):
                zps = ps_a.tile([128, S], F32, tag="a")
                for ec in range(EC):
                    nc.tensor.matmul(
                        zps[:, :FW],
                        pt[:, ec * S + st * 128 : ec * S + st * 128 + 128],
                        wo[:, ec * E + hf * FW : ec * E + (hf + 1) * FW],
                        start=(ec == 0),
                        stop=(ec == EC - 1),
                    )
                dst = z[:, st * E + hf * FW : st * E + (hf + 1) * FW]
                if hf == 0:
                    nc.scalar.mul(dst, zps[:, :FW], m_colf[:, st : st + 1])
                else:
                    nc.vector.tensor_scalar(
                        out=dst,
                        in0=zps[:, :FW],
                        scalar1=m_colf[:, st : st + 1],
                        scalar2=None,
                        op0=OP.mult,
                    )

        # y[q,f] += recip[q] * sum_k esym[k,q] z[k,f]   (+bo on head 0)
        # on the last head the drain also emits the row-sum (accum_out)
        # for the layernorm mean, and ln_qt() is pipelined in per qt.
        if nwo is not None:
            cast_wo(nwo, nwob)

        last = h == H - 1
        mh_prev = None
        for qt in range(SC):
            rc = recip_col[:, qt : qt + 1]
            mh = []
            for hf in range(FH):
                yps = ps_y.tile([128, S], F32, tag=f"y{hf}")
                for kt in range(SC):
                    nc.tensor.matmul(
                        yps[:, :FW],
                        esym[:, kt * S + qt * 128 : kt * S + qt * 128 + 128],
                        z[:, kt * E + hf * FW : kt * E + (hf + 1) * FW],
                        start=(kt == 0),
                        stop=(kt == SC - 1),
                    )
                ysl = y_sb[:, qt * E + hf * FW : qt * E + (hf + 1) * FW]
                other = bo_bc[:, hf * FW : (hf + 1) * FW] if h == 0 else ysl
                acc = None
                if last:
                    acc = statp.tile(
                        [128, 1], F32, tag="stat", name=f"mh_{qt}_{hf}"
                    )
                    mh.append(acc)
                nc.vector.scalar_tensor_tensor(
                    out=ysl,
                    in0=yps[:, :FW],
                    scalar=rc,
                    in1=other,
                    op0=OP.mult,
                    op1=OP.add,
                    accum_out=acc,
                )
            if last:
                # lag the layernorm chain one qt behind the drains so its
                # DVE/ACT ops never sit in the engine FIFOs ahead of the
                # next qt's drains (which gate PSUM bank reuse -> PE).
                if mh_prev is not None:
                    ln_qt(qt - 1, mh_prev[0], mh_prev[1])
                mh_prev = (mh[0], mh[1])
        if last:
            ln_qt(SC - 1, mh_prev[0], mh_prev[1])

    loaded = loaded0
    for h in range(H):
        with nc.named_scope(f"head{h}"):
            if h + 1 < H:
                nwh, nbh = load_wh(h + 1)
                nwo, nwob = load_wo(h + 1)
                nxt = (nwh, nbh, nwob)
            else:
                nwo = nwob = None
                nxt = None
            head(h, loaded, nwo, nwob)
            loaded = nxt

    ctx.close()


def _build_nc(trivial_gb=True):
    import concourse.bacc as bacc
    import concourse.mybir as mybir
    import concourse.tile as tile

    F32 = mybir.dt.float32
    I32 = mybir.dt.int32

    nc = bacc.Bacc("TRN2", target_bir_lowering=False, debug=False, enable_asserts=True)

    # f32r DRAM declarations: same bits as f32 (dt.np(float32r) == np.float32)
    # but lets plain HWDGE (nc.sync) DMAs feed f32r SBUF tiles without the
    # gpsimd casting path, which would serialize all weight loads on one queue.
    F32R = mybir.dt.float32r
    tensors = (
        nc.dram_tensor("x", [S, E], F32R, kind="ExternalInput"),
        nc.dram_tensor("mask", [1, S], I32, kind="ExternalInput"),
        nc.dram_tensor("wh", [H, E, E], F32R, kind="ExternalInput"),
        nc.dram_tensor("bh", [H, E], F32, kind="ExternalInput"),
        nc.dram_tensor("wo", [H, E, E], F32R, kind="ExternalInput"),
        nc.dram_tensor("bo", [1, E], F32R, kind="ExternalInput"),
        nc.dram_tensor("gamma", [1, E], F32R, kind="ExternalInput"),
        nc.dram_tensor("beta", [1, E], F32R, kind="ExternalInput"),
        nc.dram_tensor("y", [S, E], F32, kind="ExternalOutput"),
    )

    with tile.TileContext(nc) as tc:
        _emit(nc, tc, tensors, trivial_gb)

    nc.compile()
    return nc


def get_nc(trivial_gb=True):
    key = ("nc", trivial_gb)
    if key not in _CACHE:
        _CACHE[key] = _build_nc(trivial_gb)
    return _CACHE[key]


def make_in_maps(x, atten_pad_mask, Wh, bh, Wo, bo, gamma, beta):
    x = np.ascontiguousarray(np.asarray(x, dtype=np.float32))
    mask = np.ascontiguousarray(np.asarray(atten_pad_mask, dtype=np.int32))
    wh = np.ascontiguousarray(np.asarray(Wh, dtype=np.float32))
    bhv = np.ascontiguousarray(np.asarray(bh, dtype=np.float32))
    wo = np.ascontiguousarray(np.asarray(Wo, dtype=np.float32).reshape(H, E, E))
    bov = np.asarray(bo, dtype=np.float32).reshape(1, E)
    gam = np.asarray(gamma, dtype=np.float32).reshape(1, E)
    bet = np.asarray(beta, dtype=np.float32).reshape(1, E)
    return [
        {
            "x": x[b],
            "mask": mask[b],
            "wh": wh,
            "bh": bhv,
            "wo": wo,
            "bo": bov,
            "gamma": gam,
            "beta": bet,
        }
        for b in range(B)
    ]


def kernel(x, atten_pad_mask, Wh, bh, Wo, bo, gamma, beta):
    from concourse.bass_utils import run_bass_kernel_spmd

    trivial_gb = bool(
        np.all(np.asarray(gamma) == 1.0) and np.all(np.asarray(beta) == 0.0)
    )
    nc = get_nc(trivial_gb)
    in_maps = make_in_maps(x, atten_pad_mask, Wh, bh, Wo, bo, gamma, beta)
    res = run_bass_kernel_spmd(nc, in_maps, list(range(B)))
    return np.stack([res.results[b]["y"] for b in range(B)], axis=0)


# revision 39
# speedup vs baseline: 1.0524x; 1.0005x over previous
"""Multi-head self-attention (shared q/k/v projection per head) + output
projection + LayerNorm, data-parallel over batch across 8 NeuronCores.

Shapes (hardcoded): B=8, S=512, E=768, H=12.
Each core handles one batch element b: full attention for all 12 heads,
the output projection, and the final LayerNorm. No collectives; the host
scatters x/mask per batch element and concatenates the 8 outputs.

Per-core dataflow (proj matmuls fp32r; attention-side matmuls bf16 so
the stationary operand gets the compiler's fast weight load, which
floors the cadence of N<=384 matmuls at the stream rate):
  xT  = x^T                        (24 PE transposes, once)
  per head h:
    pT[e,s]    = Wh_h^T @ xT + bh  (36 MMs; bias in ACT psum->sbuf
                 drain, output cast to bf16)
    esym[k,q]  = exp(pT^T pT/sqrt(E))  UNMASKED, symmetric: only the
                 upper-triangle k-tiles are computed (free dims
                 512/384/256/128); the 6 strictly-lower 128x128 tiles
                 are PE transposes of upper ones.  The key-pad mask is
                 applied downstream (z rows + r chain), so transposed
                 tiles need no fixup.
    rT[1,q]    = m_col^T @ esym    (4 MMs; m=1-mask); transpose+recip ->
                 recip_col[q] per-partition, ready before the y drains.
    z[s,f]     = pT^T chunks @ Wo_h (+mask scale on drain: z rows for
                 masked keys are zeroed => masked keys drop out of y)
    y[q,f]    += recip[q] * (esym[.,q]^T @ z)   (+bo on head 0)
  LayerNorm(y) * gamma + beta  -> out
"""

import math
from contextlib import ExitStack

import numpy as np

B, S, E, H = 8, 512, 768, 12
EC = E // 128  # 6 chunks of e
SC = S // 128  # 4 chunks of s
FH = 2  # f halves of 384 for z/y matmuls
FW = E // FH  # 384
EPS = 1e-5
INV_SQRT_E = 1.0 / math.sqrt(E)

# scores pass kt -> first q column computed directly (free dim >= 256
# keeps fp32r at full rate; kt=3 recomputes tile (3,2) rather than
# running a 128-free matmul at 1/4 rate)
Q0 = (0, 128, 256, 384)
# lower-triangle esym tiles produced by transposing the symmetric upper
# ones: batches emitted after the exp drain their sources depend on.
T_BATCH1 = ((1, 0), (2, 0), (3, 0))  # sources (0,1),(0,2),(0,3): pass 0
T_BATCH2 = ((2, 1), (3, 1))  # sources (1,2),(1,3): pass 1
T_BATCH3 = ((3, 2),)  # source (2,3): pass 2

_CACHE = {}


def _emit(nc, tc, tensors, trivial_gb):
    import concourse.mybir as mybir

    F32 = mybir.dt.float32
    F32R = mybir.dt.float32r
    BF16 = mybir.dt.bfloat16
    I32 = mybir.dt.int32
    AF = mybir.ActivationFunctionType
    OP = mybir.AluOpType

    x_d, mask_d, wh_d, bh_d, wo_d, bo_d, gamma_d, beta_d, y_d = tensors

    ctx = ExitStack()
    pool = lambda name, bufs, **kw: ctx.enter_context(
        tc.tile_pool(name=name, bufs=bufs, **kw)
    )
    constp = pool("const", 1)
    xtp = pool("xt", 1)
    yp = pool("y", 1)
    # PSUM: 8 banks total. a=3 (proj/z chains), sc=3 (scores, transpose
    # scratch, rT, broadcasts), y0/y1 = 1 each.
    ps_a = pool("ps_a", 3, space="PSUM")
    ps_sc = pool("ps_sc", 3, space="PSUM")
    ps_y = pool("ps_y", 1, space="PSUM")

    whp = pool("wh", 2)
    wop = pool("wo", 2)
    wobp = pool("wob", 2)
    bhp = pool("bh", 2)
    ptp = pool("pt", 2)
    expp = pool("esym", 1)
    zp = pool("z", 1)
    smallp = pool("small", 2)
    statp = pool("stat", 16)
    lnp = pool("ln", 2)

    # ---- constants ----
    ident_d = nc.inline_tensor(np.eye(128, dtype=np.float32), name="ident128")
    ident = constp.tile([128, 128], F32R)
    nc.gpsimd.dma_start(ident[:], ident_d.ap())
    ident1 = constp.tile([1, 1], F32)
    nc.vector.memset(ident1[:], 1.0)
    # eps_t is produced via ACT Sqrt *after the last exp of head 11*
    # (emitted in head()): the Sqrt act-table set evicts/get evicted by
    # the Exp set, so the 1.3us table swap must land after the final Exp
    # but before the layernorm's first Sqrt -- otherwise it stalls the
    # LN chain, which backs up the DVE FIFO ahead of the y drains and
    # stalls the PE on PSUM bank reuse.
    eps_sq = constp.tile([128, 1], F32)
    nc.vector.memset(eps_sq[:], EPS * EPS)
    eps_t = constp.tile([128, 1], F32)

    # PE warmup: the HAM clock gate defaults to 1.2GHz and needs ~3.4us of
    # sustained matmul activity to release to 2.4GHz; the prologue is
    # DMA-bound, so without this the whole first head runs at half clock.
    # ~16 N=128 dummy matmuls on the identity keep the PE "busy" from
    # t~8us (ident is the first DMA to land) until real work streams in.
    warm_src = constp.tile([128, 128], F32)
    nc.vector.memset(warm_src[:], 1.0)
    warm = ps_y.tile([128, S], F32, tag="y0", name="warm")
    NWARM = 16
    for i in range(NWARM):
        nc.tensor.matmul(
            warm[:, :128],
            warm_src[:],
            warm_src[:],
            start=(i == 0),
            stop=(i == NWARM - 1),
        )


    m_col = constp.tile([128, SC], BF16)  # 1 - mask, per k-chunk column
    m_colf = constp.tile([128, SC], F32)  # same values, f32 for ACT/DVE scale
    ident_b = constp.tile([128, 128], BF16)  # for bf16 (esym) PE transposes
    nc.vector.tensor_copy(ident_b[:], ident[:])
    bo_row = constp.tile([1, E], F32R)
    gamma_bc = constp.tile([128, E], F32)
    beta_bc = constp.tile([128, E], F32)
    bo_bc = constp.tile([128, E], F32)
    ones_row_d = nc.inline_tensor(np.ones((1, 128), dtype=np.float32), name="ones_row")
    ones_row = constp.tile([1, 128], F32R)
    nc.gpsimd.dma_start(ones_row[:], ones_row_d.ap())

    xt = xtp.tile([128, EC * S], F32R)
    y_sb = yp.tile([128, SC * E], F32)

    def load_wh(h):
        wh = whp.tile([128, EC * E], F32R, tag="wh")
        nc.sync.dma_start(
            wh[:].rearrange("p (c e) -> p c e", c=EC),
            wh_d.ap()[h].rearrange("(c p) e -> p c e", p=128),
        )
        bh_t = bhp.tile([128, EC], F32, tag="bh")
        nc.sync.dma_start(bh_t[:], bh_d.ap()[h].rearrange("(c p) -> p c", p=128))
        return wh, bh_t

    def load_wo(h):
        wo = wop.tile([128, EC * E], F32R, tag="wo")
        nc.sync.dma_start(
            wo[:].rearrange("p (c e) -> p c e", c=EC),
            wo_d.ap()[h].rearrange("(c p) e -> p c e", p=128),
        )
        wob = wobp.tile([128, EC * E], BF16, tag="wob")
        return wo, wob

    def cast_wo(wo, wob):
        # f32->bf16 cast on ACT (z's moving operand must match bf16 pT):
        # two halves, ~1.9us each, emitted where ACT is otherwise idle
        half = EC * E // 2
        nc.scalar.copy(wob[:, :half], wo[:, :half])
        nc.scalar.copy(wob[:, half:], wo[:, half:])

    # ---- prologue: interleave head-0 Wh chunks with x slices on the DMA
    # queue; transposes + first proj matmuls start after ~650KB.
    wh0 = whp.tile([128, EC * E], F32R, tag="wh")
    bh0 = bhp.tile([128, EC], F32, tag="bh")
    xall = ptp.tile([128, SC * E], F32R, tag="pt", padded_shape=[128, SC * E])
    xv = xall[:].rearrange("p (t e) -> p t e", t=SC)
    for ic in range(EC):
        nc.sync.dma_start(
            xv[:, :, ic * 128 : (ic + 1) * 128],
            x_d.ap()
            .rearrange("(t p) e -> p t e", p=128)[:, :, ic * 128 : (ic + 1) * 128],
        )
        nc.sync.dma_start(
            wh0[:, ic * E : (ic + 1) * E],
            wh_d.ap()[0, ic * 128 : (ic + 1) * 128, :],
        )
    nc.sync.dma_start(bh0[:], bh_d.ap()[0].rearrange("(c p) -> p c", p=128))

    mask_i = statp.tile([128, SC], I32, tag="stat")
    nc.sync.dma_start(mask_i[:], mask_d.ap()[0].rearrange("(c p) -> p c", p=128))
    nc.vector.tensor_scalar(
        out=m_col[:], in0=mask_i[:], scalar1=-1.0, scalar2=1.0, op0=OP.mult, op1=OP.add
    )
    nc.vector.tensor_scalar(
        out=m_colf[:], in0=mask_i[:], scalar1=-1.0, scalar2=1.0, op0=OP.mult, op1=OP.add
    )
    nc.sync.dma_start(bo_row[:], bo_d.ap())
    gamma_row = lnp.tile([1, E], F32R, tag="lnt")
    nc.sync.dma_start(gamma_row[:], gamma_d.ap())
    beta_row = lnp.tile([1, E], F32R, tag="lnsq")
    nc.sync.dma_start(beta_row[:], beta_d.ap())

    wo0, wob0 = load_wo(0)

    # x transposes: 4 per e-chunk batched into one PSUM tile, one copy
    for ec in range(EC):
        trp = ps_sc.tile([128, S], F32R, tag="sc")
        for t in range(SC):
            nc.tensor.transpose(
                trp[:, t * 128 : (t + 1) * 128],
                xall[:, t * E + ec * 128 : t * E + ec * 128 + 128],
                ident[:],
            )
        nc.scalar.copy(xt[:, ec * S : (ec + 1) * S], trp[:])

    # broadcast rows to all partitions via K=1 matmuls (gamma/beta only
    # on the general path; the trivial-gb variant never reads them)
    bcast = [(bo_row, bo_bc)]
    if not trivial_gb:
        bcast += [(gamma_row, gamma_bc), (beta_row, beta_bc)]
    for row, bc in bcast:
        for f in range(FH):
            bps = ps_sc.tile([128, S], F32, tag="sc")
            nc.tensor.matmul(
                bps[:, :FW],
                ones_row[:],
                row[:, f * FW : (f + 1) * FW],
                start=True,
                stop=True,
            )
            nc.vector.tensor_copy(bc[:, f * FW : (f + 1) * FW], bps[:, :FW])

    cast_wo(wo0, wob0)

    loaded0 = (wh0, bh0, wob0)

    def proj_head0(wh, bh_t, pt):
        # ic-inner groups of 3 so the PE starts on Wh chunk 0 instead of
        # waiting for the full 2.25MB of Wh0 (prologue is DMA-bound)
        for g in range(2):
            pps = [
                ps_a.tile([128, S], F32, tag="a", name=f"pp0_{g}_{j}")
                for j in range(3)
            ]
            for ic in range(EC):
                for j in range(3):
                    et = g * 3 + j
                    nc.tensor.matmul(
                        pps[j][:],
                        wh[:, ic * E + et * 128 : ic * E + et * 128 + 128],
                        xt[:, ic * S : (ic + 1) * S],
                        start=(ic == 0),
                        stop=(ic == EC - 1),
                    )
            for j in range(3):
                et = g * 3 + j
                nc.scalar.activation(
                    pt[:, et * S : (et + 1) * S],
                    pps[j][:],
                    AF.Identity,
                    bias=bh_t[:, et : et + 1],
                    scale=1.0,
                )

    def proj(wh, bh_t, pt):
        for et in range(EC):
            pps = ps_a.tile([128, S], F32, tag="a")
            for ic in range(EC):
                nc.tensor.matmul(
                    pps[:],
                    wh[:, ic * E + et * 128 : ic * E + et * 128 + 128],
                    xt[:, ic * S : (ic + 1) * S],
                    start=(ic == 0),
                    stop=(ic == EC - 1),
                )
            nc.scalar.activation(
                pt[:, et * S : (et + 1) * S],
                pps[:],
                AF.Identity,
                bias=bh_t[:, et : et + 1],
                scale=1.0,
            )

    def transpose_batch(esym, esym_v, pairs, qlo):
        trp = ps_sc.tile([128, S], BF16, tag="sc", padded_shape=[128, 1024])
        for j, (dk, dq) in enumerate(pairs):
            nc.tensor.transpose(
                trp[:, j * 128 : (j + 1) * 128],
                esym[:, dq * S + dk * 128 : dq * S + dk * 128 + 128],
                ident_b[:],
            )
        n = len(pairs)
        k0 = pairs[0][0]
        nc.vector.tensor_copy(
            esym_v[:, k0 : k0 + n, qlo : qlo + 128],
            trp[:, : n * 128].rearrange("p (k q) -> p k q", k=n),
        )

    def ln_qt(qt, mh0, mh1):
        # layernorm of one q-tile, pipelined behind the last head's y
        # drains: mean comes free from the drains' accum_out; squares on
        # ACT, stats on DVE, the two full-width passes on the idle Pool
        # engine so nothing serializes behind the attention drains.
        ys = y_sb[:, qt * E : (qt + 1) * E]
        musum = statp.tile([128, 1], F32, tag="stat")
        nc.vector.scalar_tensor_tensor(
            out=musum[:], in0=mh0[:], scalar=0.0, in1=mh1[:], op0=OP.add, op1=OP.add
        )

        ssq = []
        for hf in range(FH):
            scr = lnp.tile([128, FW], F32, tag=("lnt", "lnsq")[hf])
            sq = statp.tile([128, 1], F32, tag="stat", name=f"ssq_{qt}_{hf}")
            nc.scalar.activation(
                scr[:],
                y_sb[:, qt * E + hf * FW : qt * E + (hf + 1) * FW],
                AF.Square,
                accum_out=sq[:],
            )
            ssq.append(sq)
        vart = statp.tile([128, 1], F32, tag="stat")
        # float immediate + two tensor reads: the TensorScalarPtr form
        # (scalar1=musum AP) costs 1.5us on DVE (per-partition pointer
        # gather on the same tile); this stt form costs ~150ns.
        nc.vector.scalar_tensor_tensor(
            out=vart[:],
            in0=musum[:],
            scalar=-1.0 / (E * E),
            in1=musum[:],
            op0=OP.mult,
            op1=OP.mult,
        )  # vart = -mu^2
        var2a = statp.tile([128, 1], F32, tag="stat")
        nc.vector.scalar_tensor_tensor(
            out=var2a[:],
            in0=ssq[0][:],
            scalar=1.0 / E,
            in1=vart[:],
            op0=OP.mult,
            op1=OP.add,
        )
        var2 = statp.tile([128, 1], F32, tag="stat")
        nc.vector.scalar_tensor_tensor(
            out=var2[:],
            in0=ssq[1][:],
            scalar=1.0 / E,
            in1=var2a[:],
            op0=OP.mult,
            op1=OP.add,
        )  # var2 = ssq/E - mu^2
        std = statp.tile([128, 1], F32, tag="stat")
        nc.scalar.activation(std[:], var2[:], AF.Sqrt, bias=eps_t[:], scale=1.0)
        rstd = statp.tile([128, 1], F32, tag="stat")
        nc.vector.reciprocal(rstd[:], std[:])
        cc = statp.tile([128, 1], F32, tag="stat")
        nc.vector.tensor_scalar(
            out=cc[:],
            in0=musum[:],
            scalar1=rstd[:],
            scalar2=-1.0 / E,
            op0=OP.mult,
            op1=OP.mult,
        )  # cc = -mu * rstd
        # o1 = (ys - mu) * rstd on ACT (per-partition scale+bias), then
        # *gamma, +beta as plain tensor-tensor passes on the idle Pool
        # engine -- keeps the big elementwise work off DVE, which is busy
        # with the last head's y drains.
        o1 = lnp.tile([128, E], F32, tag="lnt")
        nc.scalar.activation(o1[:], ys, AF.Identity, bias=cc[:], scale=rstd[:])
        if trivial_gb:
            # gamma == 1 and beta == 0 (host-checked): o1 is the output
            nc.sync.dma_start(y_d.ap()[qt * 128 : (qt + 1) * 128, :], o1[:])
            return
        t2 = lnp.tile([128, E], F32, tag="lnsq")
        nc.vector.tensor_mul(t2[:], o1[:], gamma_bc[:])
        yout = lnp.tile([128, E], F32, tag="lnyo")
        # last qt: the +beta pass is the final critical-path op -- run it
        # on DVE (~1us) instead of the slower Pool (~1.8us)
        eng = nc.vector if qt == SC - 1 else nc.gpsimd
        eng.tensor_add(yout[:], t2[:], beta_bc[:])
        nc.sync.dma_start(y_d.ap()[qt * 128 : (qt + 1) * 128, :], yout[:])

    def head(h, loaded, nwo, nwob):
        wh, bh_t, wo = loaded

        pt = ptp.tile([128, EC * S], BF16, tag="pt")
        if h == 0:
            proj_head0(wh, bh_t, pt)
        else:
            proj(wh, bh_t, pt)

        # scores (upper triangle) + exp; lower tiles by transpose
        esym = expp.tile([128, SC * S], BF16, tag="esym")
        esym_v = esym[:].rearrange("p (k q) -> p k q", k=SC)
        for kt in range(SC):
            q0 = Q0[kt]
            fw = S - q0
            scs = ps_sc.tile([128, S], F32, tag="sc")
            for ec in range(EC):
                nc.tensor.matmul(
                    scs[:, :fw],
                    pt[:, ec * S + kt * 128 : ec * S + kt * 128 + 128],
                    pt[:, ec * S + q0 : ec * S + S],
                    start=(ec == 0),
                    stop=(ec == EC - 1),
                )
            nc.scalar.activation(
                esym[:, kt * S + q0 : (kt + 1) * S],
                scs[:, :fw],
                AF.Exp,
                scale=INV_SQRT_E,
            )
            if kt == 1:
                transpose_batch(esym, esym_v, T_BATCH1, 0)
            elif kt == 2:
                transpose_batch(esym, esym_v, T_BATCH2, 128)
            elif kt == 3:
                transpose_batch(esym, esym_v, T_BATCH3, 256)

        if h == H - 1:
            # final Exp is behind us: swap in the Sqrt act-table now (off
            # the critical path) so the layernorm never waits for it
            nc.scalar.activation(eps_t[:], eps_sq[:], AF.Sqrt)

        # rT[1,q] = m^T @ esym; transpose to per-partition recip ahead of
        # the y drains so normalization never stalls the PE
        rps = ps_sc.tile([1, S], F32, tag="sc")
        for kt in range(SC):
            nc.tensor.matmul(
                rps[:],
                m_col[:, kt : kt + 1],
                esym[:, kt * S : (kt + 1) * S],
                start=(kt == 0),
                stop=(kt == SC - 1),
            )
        r_sb = smallp.tile([1, S], F32, tag="rsb")
        nc.scalar.copy(r_sb[:], rps[:])
        rtp = ps_sc.tile([128, SC], F32, tag="sc")
        for qt in range(SC):
            nc.tensor.transpose(
                rtp[:, qt : qt + 1],
                r_sb[:, qt * 128 : (qt + 1) * 128],
                ident1[:],
            )
        rsum = smallp.tile([128, SC], F32, tag="rsum")
        nc.scalar.copy(rsum[:], rtp[:])
        recip_col = smallp.tile([128, SC], F32, tag="recip")
        nc.vector.reciprocal(recip_col[:], rsum[:])

        # z[s,f] = pT^T @ Wo (+bias-free); masked key rows zeroed on drain
        z = zp.tile([128, SC * E], BF16, tag="z")
        for st in range(SC):
            for hf in range(FH):
                zps = ps_a.tile([128, S], F32, tag="a")
                for ec in range(EC):
                    nc.tensor.matmul(
                        zps[:, :FW],
                        pt[:, ec * S + st * 128 : ec * S + st * 128 + 128],
                        wo[:, ec * E + hf * FW : ec * E + (hf + 1) * FW],
                        start=(ec == 0),
                        stop=(ec == EC - 1),
                    )
                dst = z[:, st * E + hf * FW : st * E + (hf + 1) * FW]
                if hf == 0:
                    nc.scalar.mul(dst, zps[:, :FW], m_colf[:, st : st + 1])
                else:
                    nc.vector.tensor_scalar(
                        out=dst,
                        in0=zps[:, :FW],
                        scalar1=m_colf[:, st : st + 1],
                        scalar2=None,
                        op0=OP.mult,
                    )

        # y[q,f] += recip[q] * sum_k esym[k,q] z[k,f]   (+bo on head 0)
        # on the last head the drain also emits the row-sum (accum_out)
        # for the layernorm mean, and ln_qt() is pipelined in per qt.
        if nwo is not None:
            cast_wo(nwo, nwob)

        last = h == H - 1
        mh_prev = None
        for qt in range(SC):
            rc = recip_col[:, qt : qt + 1]
            mh = []
            for hf in range(FH):
                yps = ps_y.tile([128, S], F32, tag=f"y{hf}")
                for kt in range(SC):
                    nc.tensor.matmul(
                        yps[:, :FW],
                        esym[:, kt * S + qt * 128 : kt * S + qt * 128 + 128],
                        z[:, kt * E + hf * FW : kt * E + (hf + 1) * FW],
                        start=(kt == 0),
                        stop=(kt == SC - 1),
                    )
                ysl = y_sb[:, qt * E + hf * FW : qt * E + (hf + 1) * FW]
                other = bo_bc[:, hf * FW : (hf + 1) * FW] if h == 0 else ysl
                acc = None
                if last:
                    acc = statp.tile(
                        [128, 1], F32, tag="stat", name=f"mh_{qt}_{hf}"
                    )
                    mh.append(acc)
                nc.vector.scalar_tensor_tensor(
                    out=ysl,
                    in0=yps[:, :FW],
                    scalar=rc,
                    in1=other,
                    op0=OP.mult,
                    op1=OP.add,
                    accum_out=acc,
                )
            if last:
                # lag the layernorm chain one qt behind the drains so its
                # DVE/ACT ops never sit in the engine FIFOs ahead of the
                # next qt's drains (which gate PSUM bank reuse -> PE).
                if mh_prev is not None:
                    ln_qt(qt - 1, mh_prev[0], mh_prev[1])
                mh_prev = (mh[0], mh[1])
        if last:
            ln_qt(SC - 1, mh_prev[0], mh_prev[1])

    loaded = loaded0
    for h in range(H):
        with nc.named_scope(f"head{h}"):
            if h + 1 < H:
                nwh, nbh = load_wh(h + 1)
                nwo, nwob = load_wo(h + 1)
                nxt = (nwh, nbh, nwob)
            else:
                nwo = nwob = None
                nxt = None
            head(h, loaded, nwo, nwob)
            loaded = nxt

    ctx.close()


def _build_nc(trivial_gb=True):
    import concourse.bacc as bacc
    import concourse.mybir as mybir
    import concourse.tile as tile

    F32 = mybir.dt.float32
    I32 = mybir.dt.int32

    nc = bacc.Bacc("TRN2", target_bir_lowering=False, debug=False, enable_asserts=True)

    # f32r DRAM declarations: same bits as f32 (dt.np(float32r) == np.float32)
    # but lets plain HWDGE (nc.sync) DMAs feed f32r SBUF tiles without the
    # gpsimd casting path, which would serialize all weight loads on one queue.
    F32R = mybir.dt.float32r
    tensors = (
        nc.dram_tensor("x", [S, E], F32R, kind="ExternalInput"),
        nc.dram_tensor("mask", [1, S], I32, kind="ExternalInput"),
        nc.dram_tensor("wh", [H, E, E], F32R, kind="ExternalInput"),
        nc.dram_tensor("bh", [H, E], F32, kind="ExternalInput"),
        nc.dram_tensor("wo", [H, E, E], F32R, kind="ExternalInput"),
        nc.dram_tensor("bo", [1, E], F32R, kind="ExternalInput"),
        nc.dram_tensor("gamma", [1, E], F32R, kind="ExternalInput"),
        nc.dram_tensor("beta", [1, E], F32R, kind="ExternalInput"),
        nc.dram_tensor("y", [S, E], F32, kind="ExternalOutput"),
    )

    with tile.TileContext(nc) as tc:
        _emit(nc, tc, tensors, trivial_gb)

    nc.compile()
    return nc


def get_nc(trivial_gb=True):
    key = ("nc", trivial_gb)
    if key not in _CACHE:
        _CACHE[key] = _build_nc(trivial_gb)
    return _CACHE[key]


def make_in_maps(x, atten_pad_mask, Wh, bh, Wo, bo, gamma, beta):
    x = np.ascontiguousarray(np.asarray(x, dtype=np.float32))
    mask = np.ascontiguousarray(np.asarray(atten_pad_mask, dtype=np.int32))
    wh = np.ascontiguousarray(np.asarray(Wh, dtype=np.float32))
    bhv = np.ascontiguousarray(np.asarray(bh, dtype=np.float32))
    wo = np.ascontiguousarray(np.asarray(Wo, dtype=np.float32).reshape(H, E, E))
    bov = np.asarray(bo, dtype=np.float32).reshape(1, E)
    gam = np.asarray(gamma, dtype=np.float32).reshape(1, E)
    bet = np.asarray(beta, dtype=np.float32).reshape(1, E)
    return [
        {
            "x": x[b],
            "mask": mask[b],
            "wh": wh,
            "bh": bhv,
            "wo": wo,
            "bo": bov,
            "gamma": gam,
            "beta": bet,
        }
        for b in range(B)
    ]


def kernel(x, atten_pad_mask, Wh, bh, Wo, bo, gamma, beta):
    from concourse.bass_utils import run_bass_kernel_spmd

    trivial_gb = bool(
        np.all(np.asarray(gamma) == 1.0) and np.all(np.asarray(beta) == 0.0)
    )
    nc = get_nc(trivial_gb)
    in_maps = make_in_maps(x, atten_pad_mask, Wh, bh, Wo, bo, gamma, beta)
    res = run_bass_kernel_spmd(nc, in_maps, list(range(B)))
    return np.stack([res.results[b]["y"] for b in range(B)], axis=0)


# revision 42
# speedup vs baseline: 1.0677x; 1.0145x over previous
"""Multi-head self-attention (shared q/k/v projection per head) + output
projection + LayerNorm, data-parallel over batch across 8 NeuronCores.

Shapes (hardcoded): B=8, S=512, E=768, H=12.
Each core handles one batch element b: full attention for all 12 heads,
the output projection, and the final LayerNorm. No collectives; the host
scatters x/mask per batch element and concatenates the 8 outputs.

Per-core dataflow (proj matmuls fp32r; attention-side matmuls bf16 so
the stationary operand gets the compiler's fast weight load, which
floors the cadence of N<=384 matmuls at the stream rate):
  xT  = x^T                        (24 PE transposes, once)
  per head h:
    pT[e,s]    = Wh_h^T @ xT + bh  (36 MMs; bias in ACT psum->sbuf
                 drain, output cast to bf16)
    esym[k,q]  = exp(pT^T pT/sqrt(E))  UNMASKED, symmetric: only the
                 upper-triangle k-tiles are computed (free dims
                 512/384/256/128); the 6 strictly-lower 128x128 tiles
                 are PE transposes of upper ones.  The key-pad mask is
                 applied downstream (z rows + r chain), so transposed
                 tiles need no fixup.
    rT[1,q]    = m_col^T @ esym    (4 MMs; m=1-mask); transpose+recip ->
                 recip_col[q] per-partition, ready before the y drains.
    z[s,f]     = pT^T chunks @ Wo_h (+mask scale on drain: z rows for
                 masked keys are zeroed => masked keys drop out of y)
    y[q,f]    += recip[q] * (esym[.,q]^T @ z)   (+bo on head 0)
  LayerNorm(y) * gamma + beta  -> out
"""

import math
from contextlib import ExitStack

import numpy as np

B, S, E, H = 8, 512, 768, 12
EC = E // 128  # 6 chunks of e
SC = S // 128  # 4 chunks of s
FH = 2  # f halves of 384 for z/y matmuls
FW = E // FH  # 384
EPS = 1e-5
INV_SQRT_E = 1.0 / math.sqrt(E)

# scores pass kt -> first q column computed directly (free dim >= 256
# keeps fp32r at full rate; kt=3 recomputes tile (3,2) rather than
# running a 128-free matmul at 1/4 rate)
Q0 = (0, 128, 256, 384)
# lower-triangle esym tiles produced by transposing the symmetric upper
# ones: batches emitted after the exp drain their sources depend on.
T_BATCH1 = ((1, 0), (2, 0), (3, 0))  # sources (0,1),(0,2),(0,3): pass 0
T_BATCH2 = ((2, 1), (3, 1))  # sources (1,2),(1,3): pass 1
T_BATCH3 = ((3, 2),)  # source (2,3): pass 2

_CACHE = {}


def _emit(nc, tc, tensors, trivial_gb):
    import concourse.mybir as mybir

    F32 = mybir.dt.float32
    F32R = mybir.dt.float32r
    BF16 = mybir.dt.bfloat16
    I32 = mybir.dt.int32
    AF = mybir.ActivationFunctionType
    OP = mybir.AluOpType

    x_d, mask_d, wh_d, bh_d, wo_d, bo_d, gamma_d, beta_d, y_d = tensors

    ctx = ExitStack()
    pool = lambda name, bufs, **kw: ctx.enter_context(
        tc.tile_pool(name=name, bufs=bufs, **kw)
    )
    constp = pool("const", 1)
    xtp = pool("xt", 1)
    yp = pool("y", 1)
    # PSUM: 8 banks total. a=3 (proj/z chains), sc=3 (scores, transpose
    # scratch, rT, broadcasts), y0/y1 = 1 each.
    ps_a = pool("ps_a", 3, space="PSUM")
    ps_sc = pool("ps_sc", 3, space="PSUM")
    ps_y = pool("ps_y", 1, space="PSUM")

    whp = pool("wh", 2)
    wop = pool("wo", 2)
    wobp = pool("wob", 2)
    whbp = pool("whb", 2)
    bhp = pool("bh", 2)
    ptp = pool("pt", 2)
    expp = pool("esym", 1)
    zp = pool("z", 1)
    smallp = pool("small", 2)
    statp = pool("stat", 16)
    lnp = pool("ln", 2)

    # ---- constants ----
    ident_d = nc.inline_tensor(np.eye(128, dtype=np.float32), name="ident128")
    ident = constp.tile([128, 128], F32R)
    nc.gpsimd.dma_start(ident[:], ident_d.ap())
    ident1 = constp.tile([1, 1], F32)
    nc.vector.memset(ident1[:], 1.0)
    # eps_t is produced via ACT Sqrt *after the last exp of head 11*
    # (emitted in head()): the Sqrt act-table set evicts/get evicted by
    # the Exp set, so the 1.3us table swap must land after the final Exp
    # but before the layernorm's first Sqrt -- otherwise it stalls the
    # LN chain, which backs up the DVE FIFO ahead of the y drains and
    # stalls the PE on PSUM bank reuse.
    eps_sq = constp.tile([128, 1], F32)
    nc.vector.memset(eps_sq[:], EPS * EPS)
    eps_t = constp.tile([128, 1], F32)

    # PE warmup: the HAM clock gate defaults to 1.2GHz and needs ~3.4us of
    # sustained matmul activity to release to 2.4GHz; the prologue is
    # DMA-bound, so without this the whole first head runs at half clock.
    # ~16 N=128 dummy matmuls on the identity keep the PE "busy" from
    # t~8us (ident is the first DMA to land) until real work streams in.
    warm_src = constp.tile([128, 128], F32)
    nc.vector.memset(warm_src[:], 1.0)
    warm = ps_y.tile([128, S], F32, tag="y0", name="warm")
    NWARM = 10
    for i in range(NWARM):
        nc.tensor.matmul(
            warm[:, :128],
            warm_src[:],
            warm_src[:],
            start=(i == 0),
            stop=(i == NWARM - 1),
        )


    m_col = constp.tile([128, SC], BF16)  # 1 - mask, per k-chunk column
    m_colf = constp.tile([128, SC], F32)  # same values, f32 for ACT/DVE scale
    ident_b = constp.tile([128, 128], BF16)  # for bf16 (esym) PE transposes
    nc.vector.tensor_copy(ident_b[:], ident[:])
    bo_row = constp.tile([1, E], F32R)
    gamma_bc = constp.tile([128, E], F32)
    beta_bc = constp.tile([128, E], F32)
    bo_bc = constp.tile([128, E], F32)
    ones_row_d = nc.inline_tensor(np.ones((1, 128), dtype=np.float32), name="ones_row")
    ones_row = constp.tile([1, 128], F32R)
    nc.gpsimd.dma_start(ones_row[:], ones_row_d.ap())

    xt = xtp.tile([128, EC * S], BF16)
    y_sb = yp.tile([128, SC * E], F32)

    def load_wh(h):
        wh = whp.tile([128, EC * E], F32R, tag="wh")
        nc.sync.dma_start(
            wh[:].rearrange("p (c e) -> p c e", c=EC),
            wh_d.ap()[h].rearrange("(c p) e -> p c e", p=128),
        )
        whb = whbp.tile([128, EC * E], BF16, tag="whb")
        bh_t = bhp.tile([128, EC], F32, tag="bh")
        nc.sync.dma_start(bh_t[:], bh_d.ap()[h].rearrange("(c p) -> p c", p=128))
        return wh, whb, bh_t

    def load_wo(h):
        wo = wop.tile([128, EC * E], F32R, tag="wo")
        nc.sync.dma_start(
            wo[:].rearrange("p (c e) -> p c e", c=EC),
            wo_d.ap()[h].rearrange("(c p) e -> p c e", p=128),
        )
        wob = wobp.tile([128, EC * E], BF16, tag="wob")
        return wo, wob

    def cast_w(wh, whb, wo, wob):
        # f32->bf16 casts on ACT (proj/z operands must match the bf16
        # chain): four ~1.9us halves, emitted where ACT is otherwise idle
        half = EC * E // 2
        nc.scalar.copy(whb[:, :half], wh[:, :half])
        nc.scalar.copy(whb[:, half:], wh[:, half:])
        nc.scalar.copy(wob[:, :half], wo[:, :half])
        nc.scalar.copy(wob[:, half:], wo[:, half:])

    # ---- prologue: interleave head-0 Wh chunks with x slices on the DMA
    # queue; transposes + first proj matmuls start after ~650KB.
    wh0 = whp.tile([128, EC * E], F32R, tag="wh")
    whb0 = whbp.tile([128, EC * E], BF16, tag="whb")
    bh0 = bhp.tile([128, EC], F32, tag="bh")
    xall = ptp.tile([128, SC * E], F32R, tag="pt", padded_shape=[128, SC * E])
    xv = xall[:].rearrange("p (t e) -> p t e", t=SC)
    for ic in range(EC):
        nc.sync.dma_start(
            xv[:, :, ic * 128 : (ic + 1) * 128],
            x_d.ap()
            .rearrange("(t p) e -> p t e", p=128)[:, :, ic * 128 : (ic + 1) * 128],
        )
        nc.sync.dma_start(
            wh0[:, ic * E : (ic + 1) * E],
            wh_d.ap()[0, ic * 128 : (ic + 1) * 128, :],
        )
        nc.scalar.copy(
            whb0[:, ic * E : (ic + 1) * E], wh0[:, ic * E : (ic + 1) * E]
        )
    nc.sync.dma_start(bh0[:], bh_d.ap()[0].rearrange("(c p) -> p c", p=128))

    mask_i = statp.tile([128, SC], I32, tag="stat")
    nc.sync.dma_start(mask_i[:], mask_d.ap()[0].rearrange("(c p) -> p c", p=128))
    nc.vector.tensor_scalar(
        out=m_col[:], in0=mask_i[:], scalar1=-1.0, scalar2=1.0, op0=OP.mult, op1=OP.add
    )
    nc.vector.tensor_scalar(
        out=m_colf[:], in0=mask_i[:], scalar1=-1.0, scalar2=1.0, op0=OP.mult, op1=OP.add
    )
    nc.sync.dma_start(bo_row[:], bo_d.ap())
    gamma_row = lnp.tile([1, E], F32R, tag="lnt")
    nc.sync.dma_start(gamma_row[:], gamma_d.ap())
    beta_row = lnp.tile([1, E], F32R, tag="lnsq")
    nc.sync.dma_start(beta_row[:], beta_d.ap())

    wo0, wob0 = load_wo(0)
    half0 = EC * E // 2
    nc.scalar.copy(wob0[:, :half0], wo0[:, :half0])
    nc.scalar.copy(wob0[:, half0:], wo0[:, half0:])

    # x transposes: 4 per e-chunk batched into one PSUM tile, one copy
    for ec in range(EC):
        trp = ps_sc.tile([128, S], F32R, tag="sc")
        for t in range(SC):
            nc.tensor.transpose(
                trp[:, t * 128 : (t + 1) * 128],
                xall[:, t * E + ec * 128 : t * E + ec * 128 + 128],
                ident[:],
            )
        nc.scalar.copy(xt[:, ec * S : (ec + 1) * S], trp[:])

    # broadcast rows to all partitions via K=1 matmuls (gamma/beta only
    # on the general path; the trivial-gb variant never reads them)
    bcast = [(bo_row, bo_bc)]
    if not trivial_gb:
        bcast += [(gamma_row, gamma_bc), (beta_row, beta_bc)]
    for row, bc in bcast:
        for f in range(FH):
            bps = ps_sc.tile([128, S], F32, tag="sc")
            nc.tensor.matmul(
                bps[:, :FW],
                ones_row[:],
                row[:, f * FW : (f + 1) * FW],
                start=True,
                stop=True,
            )
            nc.vector.tensor_copy(bc[:, f * FW : (f + 1) * FW], bps[:, :FW])


    loaded0 = (whb0, bh0, wob0)

    def proj_head0(wh, bh_t, pt):
        # ic-inner groups of 3 so the PE starts on Wh chunk 0 instead of
        # waiting for the full 2.25MB of Wh0 (prologue is DMA-bound)
        for g in range(2):
            pps = [
                ps_a.tile([128, S], F32, tag="a", name=f"pp0_{g}_{j}")
                for j in range(3)
            ]
            for ic in range(EC):
                for j in range(3):
                    et = g * 3 + j
                    nc.tensor.matmul(
                        pps[j][:],
                        wh[:, ic * E + et * 128 : ic * E + et * 128 + 128],
                        xt[:, ic * S : (ic + 1) * S],
                        start=(ic == 0),
                        stop=(ic == EC - 1),
                    )
            for j in range(3):
                et = g * 3 + j
                nc.scalar.activation(
                    pt[:, et * S : (et + 1) * S],
                    pps[j][:],
                    AF.Identity,
                    bias=bh_t[:, et : et + 1],
                    scale=1.0,
                )

    def proj(wh, bh_t, pt):
        for et in range(EC):
            pps = ps_a.tile([128, S], F32, tag="a")
            for ic in range(EC):
                nc.tensor.matmul(
                    pps[:],
                    wh[:, ic * E + et * 128 : ic * E + et * 128 + 128],
                    xt[:, ic * S : (ic + 1) * S],
                    start=(ic == 0),
                    stop=(ic == EC - 1),
                )
            nc.scalar.activation(
                pt[:, et * S : (et + 1) * S],
                pps[:],
                AF.Identity,
                bias=bh_t[:, et : et + 1],
                scale=1.0,
            )

    def transpose_batch(esym, esym_v, pairs, qlo):
        trp = ps_sc.tile([128, S], BF16, tag="sc", padded_shape=[128, 1024])
        for j, (dk, dq) in enumerate(pairs):
            nc.tensor.transpose(
                trp[:, j * 128 : (j + 1) * 128],
                esym[:, dq * S + dk * 128 : dq * S + dk * 128 + 128],
                ident_b[:],
            )
        n = len(pairs)
        k0 = pairs[0][0]
        nc.vector.tensor_copy(
            esym_v[:, k0 : k0 + n, qlo : qlo + 128],
            trp[:, : n * 128].rearrange("p (k q) -> p k q", k=n),
        )

    def ln_qt(qt, mh0, mh1):
        # layernorm of one q-tile, pipelined behind the last head's y
        # drains: mean comes free from the drains' accum_out; squares on
        # ACT, stats on DVE, the two full-width passes on the idle Pool
        # engine so nothing serializes behind the attention drains.
        ys = y_sb[:, qt * E : (qt + 1) * E]
        musum = statp.tile([128, 1], F32, tag="stat")
        nc.vector.scalar_tensor_tensor(
            out=musum[:], in0=mh0[:], scalar=0.0, in1=mh1[:], op0=OP.add, op1=OP.add
        )

        ssq = []
        for hf in range(FH):
            scr = lnp.tile([128, FW], F32, tag=("lnt", "lnsq")[hf])
            sq = statp.tile([128, 1], F32, tag="stat", name=f"ssq_{qt}_{hf}")
            nc.scalar.activation(
                scr[:],
                y_sb[:, qt * E + hf * FW : qt * E + (hf + 1) * FW],
                AF.Square,
                accum_out=sq[:],
            )
            ssq.append(sq)
        vart = statp.tile([128, 1], F32, tag="stat")
        # float immediate + two tensor reads: the TensorScalarPtr form
        # (scalar1=musum AP) costs 1.5us on DVE (per-partition pointer
        # gather on the same tile); this stt form costs ~150ns.
        nc.vector.scalar_tensor_tensor(
            out=vart[:],
            in0=musum[:],
            scalar=-1.0 / (E * E),
            in1=musum[:],
            op0=OP.mult,
            op1=OP.mult,
        )  # vart = -mu^2
        var2a = statp.tile([128, 1], F32, tag="stat")
        nc.vector.scalar_tensor_tensor(
            out=var2a[:],
            in0=ssq[0][:],
            scalar=1.0 / E,
            in1=vart[:],
            op0=OP.mult,
            op1=OP.add,
        )
        var2 = statp.tile([128, 1], F32, tag="stat")
        nc.vector.scalar_tensor_tensor(
            out=var2[:],
            in0=ssq[1][:],
            scalar=1.0 / E,
            in1=var2a[:],
            op0=OP.mult,
            op1=OP.add,
        )  # var2 = ssq/E - mu^2
        std = statp.tile([128, 1], F32, tag="stat")
        nc.scalar.activation(std[:], var2[:], AF.Sqrt, bias=eps_t[:], scale=1.0)
        rstd = statp.tile([128, 1], F32, tag="stat")
        nc.vector.reciprocal(rstd[:], std[:])
        cc = statp.tile([128, 1], F32, tag="stat")
        nc.vector.tensor_scalar(
            out=cc[:],
            in0=musum[:],
            scalar1=rstd[:],
            scalar2=-1.0 / E,
            op0=OP.mult,
            op1=OP.mult,
        )  # cc = -mu * rstd
        # o1 = (ys - mu) * rstd on ACT (per-partition scale+bias), then
        # *gamma, +beta as plain tensor-tensor passes on the idle Pool
        # engine -- keeps the big elementwise work off DVE, which is busy
        # with the last head's y drains.
        o1 = lnp.tile([128, E], F32, tag="lnt")
        nc.scalar.activation(o1[:], ys, AF.Identity, bias=cc[:], scale=rstd[:])
        if trivial_gb:
            # gamma == 1 and beta == 0 (host-checked): o1 is the output
            nc.sync.dma_start(y_d.ap()[qt * 128 : (qt + 1) * 128, :], o1[:])
            return
        t2 = lnp.tile([128, E], F32, tag="lnsq")
        nc.vector.tensor_mul(t2[:], o1[:], gamma_bc[:])
        yout = lnp.tile([128, E], F32, tag="lnyo")
        # last qt: the +beta pass is the final critical-path op -- run it
        # on DVE (~1us) instead of the slower Pool (~1.8us)
        eng = nc.vector if qt == SC - 1 else nc.gpsimd
        eng.tensor_add(yout[:], t2[:], beta_bc[:])
        nc.sync.dma_start(y_d.ap()[qt * 128 : (qt + 1) * 128, :], yout[:])

    def head(h, loaded, casts):
        wh, bh_t, wo = loaded

        pt = ptp.tile([128, EC * S], BF16, tag="pt")
        if h == 0:
            proj_head0(wh, bh_t, pt)
        else:
            proj(wh, bh_t, pt)

        # scores (upper triangle) + exp; lower tiles by transpose
        esym = expp.tile([128, SC * S], BF16, tag="esym")
        esym_v = esym[:].rearrange("p (k q) -> p k q", k=SC)
        for kt in range(SC):
            q0 = Q0[kt]
            fw = S - q0
            scs = ps_sc.tile([128, S], F32, tag="sc")
            for ec in range(EC):
                nc.tensor.matmul(
                    scs[:, :fw],
                    pt[:, ec * S + kt * 128 : ec * S + kt * 128 + 128],
                    pt[:, ec * S + q0 : ec * S + S],
                    start=(ec == 0),
                    stop=(ec == EC - 1),
                )
            nc.scalar.activation(
                esym[:, kt * S + q0 : (kt + 1) * S],
                scs[:, :fw],
                AF.Exp,
                scale=INV_SQRT_E,
            )
            if kt == 1:
                transpose_batch(esym, esym_v, T_BATCH1, 0)
            elif kt == 2:
                transpose_batch(esym, esym_v, T_BATCH2, 128)
            elif kt == 3:
                transpose_batch(esym, esym_v, T_BATCH3, 256)

        if h == H - 1:
            # final Exp is behind us: swap in the Sqrt act-table now (off
            # the critical path) so the layernorm never waits for it
            nc.scalar.activation(eps_t[:], eps_sq[:], AF.Sqrt)

        # rT[1,q] = m^T @ esym; transpose to per-partition recip ahead of
        # the y drains so normalization never stalls the PE
        rps = ps_sc.tile([1, S], F32, tag="sc")
        for kt in range(SC):
            nc.tensor.matmul(
                rps[:],
                m_col[:, kt : kt + 1],
                esym[:, kt * S : (kt + 1) * S],
                start=(kt == 0),
                stop=(kt == SC - 1),
            )
        r_sb = smallp.tile([1, S], F32, tag="rsb")
        nc.scalar.copy(r_sb[:], rps[:])
        rtp = ps_sc.tile([128, SC], F32, tag="sc")
        for qt in range(SC):
            nc.tensor.transpose(
                rtp[:, qt : qt + 1],
                r_sb[:, qt * 128 : (qt + 1) * 128],
                ident1[:],
            )
        rsum = smallp.tile([128, SC], F32, tag="rsum")
        nc.scalar.copy(rsum[:], rtp[:])
        recip_col = smallp.tile([128, SC], F32, tag="recip")
        nc.vector.reciprocal(recip_col[:], rsum[:])

        # z[s,f] = pT^T @ Wo (+bias-free); masked key rows zeroed on drain
        z = zp.tile([128, SC * E], BF16, tag="z")
        for st in range(SC):
            for hf in range(FH):
                zps = ps_a.tile([128, S], F32, tag="a")
                for ec in range(EC):
                    nc.tensor.matmul(
                        zps[:, :FW],
                        pt[:, ec * S + st * 128 : ec * S + st * 128 + 128],
                        wo[:, ec * E + hf * FW : ec * E + (hf + 1) * FW],
                        start=(ec == 0),
                        stop=(ec == EC - 1),
                    )
                dst = z[:, st * E + hf * FW : st * E + (hf + 1) * FW]
                if hf == 0:
                    nc.scalar.mul(dst, zps[:, :FW], m_colf[:, st : st + 1])
                else:
                    nc.vector.tensor_scalar(
                        out=dst,
                        in0=zps[:, :FW],
                        scalar1=m_colf[:, st : st + 1],
                        scalar2=None,
                        op0=OP.mult,
                    )

        # y[q,f] += recip[q] * sum_k esym[k,q] z[k,f]   (+bo on head 0)
        # on the last head the drain also emits the row-sum (accum_out)
        # for the layernorm mean, and ln_qt() is pipelined in per qt.
        if casts is not None:
            cast_w(*casts)

        last = h == H - 1
        mh_prev = None
        for qt in range(SC):
            rc = recip_col[:, qt : qt + 1]
            mh = []
            for hf in range(FH):
                yps = ps_y.tile([128, S], F32, tag=f"y{hf}")
                for kt in range(SC):
                    nc.tensor.matmul(
                        yps[:, :FW],
                        esym[:, kt * S + qt * 128 : kt * S + qt * 128 + 128],
                        z[:, kt * E + hf * FW : kt * E + (hf + 1) * FW],
                        start=(kt == 0),
                        stop=(kt == SC - 1),
                    )
                ysl = y_sb[:, qt * E + hf * FW : qt * E + (hf + 1) * FW]
                other = bo_bc[:, hf * FW : (hf + 1) * FW] if h == 0 else ysl
                acc = None
                if last:
                    acc = statp.tile(
                        [128, 1], F32, tag="stat", name=f"mh_{qt}_{hf}"
                    )
                    mh.append(acc)
                nc.vector.scalar_tensor_tensor(
                    out=ysl,
                    in0=yps[:, :FW],
                    scalar=rc,
                    in1=other,
                    op0=OP.mult,
                    op1=OP.add,
                    accum_out=acc,
                )
            if last:
                # lag the layernorm chain one qt behind the drains so its
                # DVE/ACT ops never sit in the engine FIFOs ahead of the
                # next qt's drains (which gate PSUM bank reuse -> PE).
                if mh_prev is not None:
                    ln_qt(qt - 1, mh_prev[0], mh_prev[1])
                mh_prev = (mh[0], mh[1])
        if last:
            ln_qt(SC - 1, mh_prev[0], mh_prev[1])

    loaded = loaded0
    for h in range(H):
        with nc.named_scope(f"head{h}"):
            if h + 1 < H:
                nwh, nwhb, nbh = load_wh(h + 1)
                nwo, nwob = load_wo(h + 1)
                nxt = (nwhb, nbh, nwob)
                casts = (nwh, nwhb, nwo, nwob)
            else:
                casts = None
                nxt = None
            head(h, loaded, casts)
            loaded = nxt

    ctx.close()


def _build_nc(trivial_gb=True):
    import concourse.bacc as bacc
    import concourse.mybir as mybir
    import concourse.tile as tile

    F32 = mybir.dt.float32
    I32 = mybir.dt.int32

    nc = bacc.Bacc("TRN2", target_bir_lowering=False, debug=False, enable_asserts=True)

    # f32r DRAM declarations: same bits as f32 (dt.np(float32r) == np.float32)
    # but lets plain HWDGE (nc.sync) DMAs feed f32r SBUF tiles without the
    # gpsimd casting path, which would serialize all weight loads on one queue.
    F32R = mybir.dt.float32r
    tensors = (
        nc.dram_tensor("x", [S, E], F32R, kind="ExternalInput"),
        nc.dram_tensor("mask", [1, S], I32, kind="ExternalInput"),
        nc.dram_tensor("wh", [H, E, E], F32R, kind="ExternalInput"),
        nc.dram_tensor("bh", [H, E], F32, kind="ExternalInput"),
        nc.dram_tensor("wo", [H, E, E], F32R, kind="ExternalInput"),
        nc.dram_tensor("bo", [1, E], F32R, kind="ExternalInput"),
        nc.dram_tensor("gamma", [1, E], F32R, kind="ExternalInput"),
        nc.dram_tensor("beta", [1, E], F32R, kind="ExternalInput"),
        nc.dram_tensor("y", [S, E], F32, kind="ExternalOutput"),
    )

    with tile.TileContext(nc) as tc:
        _emit(nc, tc, tensors, trivial_gb)

    nc.compile()
    return nc


def get_nc(trivial_gb=True):
    key = ("nc", trivial_gb)
    if key not in _CACHE:
        _CACHE[key] = _build_nc(trivial_gb)
    return _CACHE[key]


def make_in_maps(x, atten_pad_mask, Wh, bh, Wo, bo, gamma, beta):
    x = np.ascontiguousarray(np.asarray(x, dtype=np.float32))
    mask = np.ascontiguousarray(np.asarray(atten_pad_mask, dtype=np.int32))
    wh = np.ascontiguousarray(np.asarray(Wh, dtype=np.float32))
    bhv = np.ascontiguousarray(np.asarray(bh, dtype=np.float32))
    wo = np.ascontiguousarray(np.asarray(Wo, dtype=np.float32).reshape(H, E, E))
    bov = np.asarray(bo, dtype=np.float32).reshape(1, E)
    gam = np.asarray(gamma, dtype=np.float32).reshape(1, E)
    bet = np.asarray(beta, dtype=np.float32).reshape(1, E)
    return [
        {
            "x": x[b],
            "mask": mask[b],
            "wh": wh,
            "bh": bhv,
            "wo": wo,
            "bo": bov,
            "gamma": gam,
            "beta": bet,
        }
        for b in range(B)
    ]


def kernel(x, atten_pad_mask, Wh, bh, Wo, bo, gamma, beta):
    from concourse.bass_utils import run_bass_kernel_spmd

    trivial_gb = bool(
        np.all(np.asarray(gamma) == 1.0) and np.all(np.asarray(beta) == 0.0)
    )
    nc = get_nc(trivial_gb)
    in_maps = make_in_maps(x, atten_pad_mask, Wh, bh, Wo, bo, gamma, beta)
    res = run_bass_kernel_spmd(nc, in_maps, list(range(B)))
    return np.stack([res.results[b]["y"] for b in range(B)], axis=0)
